# revision 50
# baseline (speedup 1.0000x reference)
"""Trainium2 Bass kernel for nn_Net_17532056502451.

5 "think" iterations: shift-window cosine selector (159 shifts) + softmax
attention + scatter-back + conv-style encoder/decoder with energy argmax
(81 shifts), masked-MSE losses averaged.  Data-parallel: 1024 tokens over
8 cores, 128 tokens/core (one per SBUF partition), token-major.

v2 mappings per core:
- dot correlation: ONE bf16 tensor_tensor (2 elem/cyc) building all 12720
  products + bf16 tree adds (80->40->20->10->5) + fp32 tensor_reduce.
- sliding norms: Square + prefix-scan + strided diff (fp32).
- per-token dynamic windows: GPSIMD ap_gather (idx[p,j] = off_p + j, 16
  candidate lanes) + mask-mult + tensor_reduce diagonal extract (2 DVE ops
  instead of 16 predicated copies).
- energy: quadratic Gram form, all-bf16 PE pipeline: z built in one DVE op,
  transposed on PE (bf16, 1 cyc/row), PSUM->SBUF copies batched 4 chunks at
  a time, E matmuls flipped (z chunk stationary, Az moving) so E lands
  token-major in PSUM - no transpose-back.
- encoder: h-major H matmuls with b_enc folded in via a ones row;
  decoder: X matmuls flipped token-major with b_src via a k=1 matmul -
  no back-transposes.
- softmax exp + denominator in one scalar-engine op (accum_out); loss
  sum-of-squares via scalar Square + accum_out.
"""
import numpy as np

IDIM = 80
ODIM = 80
HDIM = 512
THINK_ITER = 5
TEMPER = 0.7
B, T = 4, 256
NTOK = B * T
P = 128
NCORES = 8
S1 = 159
S2 = 81
NFEAT = 80 * 80
NCHUNK = NFEAT // 128   # 50

_cache = {}


def _bf16(a):
    import ml_dtypes
    return np.asarray(a, dtype=ml_dtypes.bfloat16)


def _get_macscan():
    """Custom DVE op: out[p,k] = running sum of in0[p,:k+1]*in1[p,:k+1].

    Registered at runtime (documented extension point: per-NEFF DVE table).
    One full-rate pass computes all windowed dot products; window sums are
    recovered by differencing page-end prefix values.
    """
    import concourse.dve_ops as dve_ops
    from concourse.dve_ops import DveOp, OPS, DveOpSpec
    from concourse.dve_spec import Spec, Src0, Src1, scan, lower
    from concourse.dve_spec import AluOp as SAlu
    from concourse.dve_spec import _has_src1 as has_src1

    for o in OPS:
        if o.name == "MACSCAN_ANT":
            return o

    def ref(in0, in1):
        p = in0.shape[0]
        prod = (np.asarray(in0, np.float64) * np.asarray(in1, np.float64))
        return np.cumsum(prod.reshape(p, -1), axis=1).reshape(
            in0.shape).astype(np.float32)

    sp = Spec(body=scan(SAlu.ADD, Src0 * Src1), reference=ref)
    row = max(dve_ops._SUB_OPCODE_FOR_NAME.values()) + 1
    dve_ops._SUB_OPCODE_FOR_NAME["MACSCAN_ANT"] = row
    shas = {}
    for ver in ("v3", "v4"):
        tmp = DveOpSpec(name="MACSCAN_ANT", opcode=row,
                        uops=lower(sp, ver=ver), rd1_en=has_src1(sp))
        shas[ver] = tmp.sha(ver)
    op = DveOp("MACSCAN_ANT", sp, subdim=False, uops_sha=shas)
    OPS.append(op)
    dve_ops.CUSTOM_DVE_SPECS[op.name] = op.spec
    return op


def _build_consts(W_enc, b_enc, W_src, b_src):
    W_enc = np.asarray(W_enc, np.float32)
    b_enc = np.asarray(b_enc, np.float32)
    W_src = np.asarray(W_src, np.float32)
    b_src = np.asarray(b_src, np.float32)
    C = (W_enc.T @ W_enc).astype(np.float32)
    q = (W_enc.T @ b_enc).astype(np.float32)
    bb = np.float32(b_enc @ b_enc)
    # E[t,s] = sum_{d,i} Az[s, 80d+i] * ye_i ye_{i+d} + sum_i 2 q[dd+i] ye_i
    #          + bb,  dd = 80 - s
    Az = np.zeros((S2, NFEAT), np.float32)
    Al = np.zeros((S2, 80), np.float32)
    for s in range(S2):
        dd = 80 - s
        blk = C[dd:dd + 80, dd:dd + 80]
        for d in range(80):
            diag = np.diagonal(blk, offset=d).copy()
            Az[s, d * 80: d * 80 + (80 - d)] = (2.0 if d > 0 else 1.0) * diag
        Al[s, :80] = 2.0 * q[dd:dd + 80]
        # bb (constant across s and t) dropped: argmax-invariant shift
    Az_cat = np.ascontiguousarray(Az.T)               # (6400, 81): pi-major
    Atail = np.ascontiguousarray(Al.T)                # (80, 81)
    W_encT = np.ascontiguousarray(W_enc.T)            # (160, 512)
    We1 = W_encT[0:128]                               # (128, 512)
    We2 = np.zeros((33, 512), np.float32)             # 32 c's + bias row
    We2[0:32] = W_encT[128:160]
    We2[32] = b_enc
    W_srcT = np.ascontiguousarray(W_src.T)            # (512, 160)
    iota8 = np.broadcast_to(np.arange(8, dtype=np.int32), (P, 8)).copy()
    ident = np.eye(128, dtype=np.float32)
    return dict(Az=_bf16(Az_cat), Atail=_bf16(Atail),
                We1=_bf16(We1), We2=_bf16(We2), Ws=_bf16(W_srcT),
                bsrc=_bf16(b_src.reshape(1, 160)),
                ones1=_bf16(np.ones((1, 128), np.float32)),
                ident=_bf16(ident), iota8=iota8)


def _build_nc():
    import concourse.bass as bass
    import concourse.bacc as bacc
    import concourse.mybir as mybir
    from concourse.tile import TileContext

    F32 = mybir.dt.float32
    BF16 = mybir.dt.bfloat16
    I16 = mybir.dt.int16
    U32 = mybir.dt.uint32
    Op = mybir.AluOpType
    AF = mybir.ActivationFunctionType
    AX = mybir.AxisListType

    nc = bacc.Bacc()
    d_x = nc.declare_dram_parameter("x", [P, 80], F32, isOutput=False)
    d_y = nc.declare_dram_parameter("y", [P, 80], F32, isOutput=False)
    d_A = nc.declare_dram_parameter("Az", [NFEAT, 81], BF16, isOutput=False)
    d_At = nc.declare_dram_parameter("Atail", [80, 81], BF16, isOutput=False)
    d_We1 = nc.declare_dram_parameter("We1", [128, 512], BF16, isOutput=False)
    d_We2 = nc.declare_dram_parameter("We2", [33, 512], BF16, isOutput=False)
    d_Ws = nc.declare_dram_parameter("Ws", [512, 160], BF16, isOutput=False)
    d_bs = nc.declare_dram_parameter("bsrc", [1, 160], BF16, isOutput=False)
    d_on = nc.declare_dram_parameter("ones1", [1, 128], BF16, isOutput=False)
    d_id = nc.declare_dram_parameter("ident", [128, 128], BF16, isOutput=False)
    d_i8 = nc.declare_dram_parameter("iota8", [P, 8], mybir.dt.int32,
                                     isOutput=False)
    d_out = nc.declare_dram_parameter("losspart", [P, 8], F32, isOutput=True)

    with TileContext(nc) as tc:
        with (
            tc.tile_pool(name="const", bufs=1) as cpool,
            tc.tile_pool(name="work", bufs=1) as pool,
            tc.tile_pool(name="ps_rot", bufs=2, space="PSUM") as pp,
            tc.tile_pool(name="ps_e", bufs=1, space="PSUM") as ppe,
            tc.tile_pool(name="ps_h", bufs=1, space="PSUM") as pph,
            tc.tile_pool(name="ps_x", bufs=1, space="PSUM") as ppx,
        ):
            # ---- constants ----
            A_t = cpool.tile([P, NCHUNK * 81], BF16, tag="A")
            for k in range(NCHUNK):
                nc.sync.dma_start(A_t[:, k * 81:(k + 1) * 81],
                                  d_A[k * 128:(k + 1) * 128, :])
            At_t = cpool.tile([80, 81], BF16, tag="At")
            nc.sync.dma_start(At_t[:], d_At[:])
            We1_t = cpool.tile([128, 512], BF16, tag="We1")
            nc.sync.dma_start(We1_t[:], d_We1[:])
            We2_t = cpool.tile([33, 512], BF16, tag="We2")
            nc.sync.dma_start(We2_t[:], d_We2[:])
            Ws_t = cpool.tile([P, 4 * 160], BF16, tag="Ws")
            for k in range(4):
                nc.sync.dma_start(Ws_t[:, k * 160:(k + 1) * 160],
                                  d_Ws[k * 128:(k + 1) * 128, :])
            bs_t = cpool.tile([1, 160], BF16, tag="bs")
            nc.sync.dma_start(bs_t[:], d_bs[:])
            on_t = cpool.tile([1, 128], BF16, tag="on")
            nc.sync.dma_start(on_t[:], d_on[:])
            id_t = cpool.tile([128, 128], BF16, tag="id")
            nc.sync.dma_start(id_t[:], d_id[:])
            i8_t = cpool.tile([P, 8], mybir.dt.int32, tag="i8")
            nc.sync.dma_start(i8_t[:], d_i8[:])

            # ---- state ----
            xpad = pool.tile([P, 238], F32, tag="xpad")
            yres = pool.tile([P, 80], F32, tag="yres")
            keep = pool.tile([P, 80], F32, tag="keep")
            yap = pool.tile([P, 240], F32, tag="yap")
            lossp = pool.tile([P, 8], F32, tag="lossp")
            yah = pool.tile([P, 160], BF16, tag="yah")
            nc.vector.memset(xpad[:], 0.0)
            nc.vector.memset(yap[:], 0.0)
            nc.vector.memset(lossp[:], 0.0)
            nc.sync.dma_start(xpad[:, 79:159], d_x[:])
            nc.sync.dma_start(yres[:], d_y[:])
            nc.vector.tensor_scalar(keep[:], yres[:], 0.0, None, Op.not_equal)

            # dot workspace
            dscan = pool.tile([P, S1 * 80], F32, tag="dscan")
            dot = pool.tile([P, S1], F32, tag="dot")
            adot = pool.tile([P, S1], F32, tag="adot")
            gsel = pool.tile([P, S1], F32, tag="gsel")
            cs2 = pool.tile([P, 238], F32, tag="cs2")
            nsq = pool.tile([P, S1], F32, tag="nsq")
            rnsq = pool.tile([P, S1], F32, tag="rnsq")

            I32 = mybir.dt.int32
            mx8 = pool.tile([P, 8], F32, tag="mx8")
            mi8 = pool.tile([P, 8], U32, tag="mi8")
            thf = pool.tile([P, 1], F32, tag="thf")
            thxf = pool.tile([P, 1], F32, tag="thxf")
            sff = pool.tile([P, 1], F32, tag="sff")
            dff = pool.tile([P, 1], F32, tag="dff")
            offI = pool.tile([P, 1], I32, tag="offI")
            bits = pool.tile([P, 8], I32, tag="bits")
            onesI = pool.tile([P, 8], I32, tag="onesI")
            nc.vector.memset(onesI[:], 1)
            wb1 = pool.tile([P, 240], F32, tag="wb1")
            wb2 = pool.tile([P, 240], F32, tag="wb2")
            wb3 = pool.tile([P, 240], F32, tag="wb3")
            yhath = pool.tile([P, 160], BF16, tag="yhath")
            zt = pool.tile([P, 80], F32, tag="zt")
            et = pool.tile([P, 80], F32, tag="et")
            ssum = pool.tile([P, 1], F32, tag="ssum")
            rsum = pool.tile([P, 1], F32, tag="rsum")
            nzm = pool.tile([P, 1], F32, tag="nzm")

            zfeat = pool.tile([P, NFEAT], BF16, tag="zfeat")
            zsb = pool.tile([P, NFEAT], BF16, tag="zsb")
            etl = pool.tile([128, 128], BF16, tag="etl")
            Etok = pool.tile([P, S2], F32, tag="Etok")
            yh0 = pool.tile([128, 128], BF16, tag="yh0")
            yh1 = pool.tile([33, 128], BF16, tag="yh1")
            nc.sync.dma_start(yh1[32:33, :], d_on[:])
            hsb = pool.tile([128, 512], BF16, tag="hsb")
            xext = pool.tile([P, 160], F32, tag="xext")
            dtmp = pool.tile([P, 80], F32, tag="dtmp")
            dsq = pool.tile([P, 80], F32, tag="dsq")

            macscan = _cache["macscan"]

            def sliding_norms():
                # cs2[j] = sum_{m<=j} xpad[m]^2; nsq[s] = cs2[s+79]-cs2[s-1]
                nc.vector._custom_dve(macscan, out=cs2[:], in0=xpad[:],
                                      in1=xpad[:])
                nc.vector.tensor_tensor(nsq[:, 1:159], cs2[:, 80:238],
                                        cs2[:, 0:158], Op.subtract)
                nc.vector.tensor_copy(nsq[:, 0:1], cs2[:, 79:80])

            sliding_norms()

            def barrel(src, wb, out_w, maxoff):
                """wb[p, j] <- src[p, off_p + j] for j in [0, out_w).

                bits must already hold (off >> b) & 1.  In-place left shifts:
                per-partition masks mean untouched rows keep their values;
                shifted reads are ahead of writes (stream order) so in-place
                is safe.  src=None means wb was pre-loaded by the caller.
                """
                nbits = (maxoff).bit_length()
                cw = out_w + maxoff
                if src is not None:
                    nc.vector.tensor_copy(wb[:, 0:cw], src[:, 0:cw])
                for b in range(nbits - 1, -1, -1):
                    sh = 1 << b
                    w = out_w + min(maxoff, 2 * sh - 1) - sh
                    nc.vector.copy_predicated(
                        wb[:, 0:w], bits[:, b:b + 1].to_broadcast((P, w)),
                        wb[:, sh:sh + w])

            def mkbits(src_i32):
                nc.vector.tensor_tensor(
                    bits[:], src_i32[:, 0:1].to_broadcast((P, 8)), i8_t[:],
                    Op.logical_shift_right)
                nc.vector.tensor_tensor(bits[:], bits[:], onesI[:],
                                        Op.bitwise_and)

            for it in range(THINK_ITER):
                # --- dot[p,s] = sum_c xpad[p,s+c]*yres[p,c]: one MAC-scan,
                # windowed sums recovered by differencing page ends ---
                in0 = bass.AP(xpad.tensor, xpad.offset,
                              [list(xpad.ap[0]), [1, S1], [1, 80]])
                in1 = bass.AP(yres.tensor, yres.offset,
                              [list(yres.ap[0]), [0, S1], [1, 80]])
                dview = dscan[:].rearrange("p (s c) -> p s c", c=80)
                nc.vector._custom_dve(macscan, out=dview, in0=in0, in1=in1)
                pe0 = bass.AP(dscan.tensor, dscan.offset + 79, [list(dscan.ap[0]), [80, 158]])
                pe1 = bass.AP(dscan.tensor, dscan.offset + 159, [list(dscan.ap[0]), [80, 158]])
                nc.vector.tensor_tensor(dot[:, 1:159], pe1, pe0, Op.subtract)
                nc.vector.tensor_copy(dot[:, 0:1], dscan[:, 79:80])
                # --- theta = argmax dot*|dot|/nsq ---
                nc.vector.tensor_scalar_max(rnsq[:], nsq[:], 1e-30)
                nc.scalar.activation(adot[:], dot[:], AF.Abs)
                nc.vector.reciprocal(rnsq[:], rnsq[:])
                nc.vector.tensor_tensor(gsel[:], dot[:], adot[:], Op.mult)
                nc.vector.tensor_tensor(gsel[:], gsel[:], rnsq[:], Op.mult)
                nc.vector.max(mx8[:], gsel[:])
                nc.vector.max_index(mi8[:], mx8[:], gsel[:])
                nc.vector.tensor_copy(thf[:], mi8[:, 0:1])
                # --- y_align barrel gather: yal = wb1[:, 0:80] ---
                nc.vector.tensor_copy(offI[:], mi8[:, 0:1])
                mkbits(offI)
                barrel(xpad, wb1, 80, 158)
                yal = wb1
                # --- softmax attention -> y_att in yap[:, 80:160] ---
                nc.vector.tensor_tensor(zt[:], yal[:, 0:80], yres[:], Op.mult)
                nc.vector.max(mx8[:], zt[:])
                nc.vector.tensor_scalar_mul(nzm[:], mx8[:, 0:1], -1.0 / TEMPER)
                nc.scalar.activation(et[:], zt[:], AF.Exp, bias=nzm[:, 0:1],
                                     scale=1.0 / TEMPER,
                                     accum_out=ssum[:, 0:1])
                nc.vector.reciprocal(rsum[:], ssum[:])
                nc.vector.scalar_tensor_tensor(yap[:, 80:160], et[:],
                                               rsum[:, 0:1], yal[:, 0:80],
                                               Op.mult, Op.mult)
                nc.scalar.copy(yah[:], yap[:, 80:240])
                # --- z features z[p, 80d+i] = ye[i]*ye[i+d] (one op) ---
                zin0 = bass.AP(yah.tensor, yah.offset,
                               [list(yah.ap[0]), [0, 80], [1, 80]])
                zin1 = bass.AP(yah.tensor, yah.offset,
                               [list(yah.ap[0]), [1, 80], [1, 80]])
                zv = zfeat[:].rearrange("p (d i) -> p d i", i=80)
                nc.vector.tensor_tensor(zv, zin0, zin1, Op.mult)
                # --- tail features: ya^T via DMA xbar transpose ---
                nc.sync.dma_start_transpose(etl[:], yah[:, 0:128])
                # --- E: z transposed by DMA xbar (halves), flipped MMs ---
                Eps = ppe.tile([P, S2], mybir.dt.float32, tag="Eps")
                half = NCHUNK // 2
                for h in range(2):
                    k0, k1 = h * half, (h + 1) * half
                    zo = zsb[:, k0 * 128:k1 * 128].rearrange(
                        "p (k t) -> p k t", t=128)
                    nc.sync.dma_start_transpose(
                        zo, zfeat[:, k0 * 128:k1 * 128])
                    for k in range(k0, k1):
                        nc.tensor.matmul(
                            Eps[:], zsb[:, k * 128:(k + 1) * 128],
                            A_t[:, k * 81:(k + 1) * 81],
                            start=(k == 0), stop=False)
                nc.tensor.matmul(Eps[:], etl[0:80, :], At_t[:], start=False,
                                 stop=True)
                # --- x_ele gather (overlaps E pipe): off = 159 - theta ---
                nc.vector.tensor_scalar(thxf[:], thf[:], -1.0, 159.0,
                                        Op.mult, Op.add)
                nc.vector.tensor_copy(offI[:], thxf[:])
                mkbits(offI)
                barrel(yap, wb2, 80, 159)
                nc.vector.tensor_tensor(xpad[:, 79:159], xpad[:, 79:159],
                                        wb2[:, 0:80], Op.subtract)
                # next-iter sliding norms (overlaps E pipe)
                sliding_norms()
                # --- s* = argmax E (token-major PSUM -> SBUF), d* = 80-s* ---
                nc.scalar.copy(Etok[:], Eps[:])
                nc.vector.max(mx8[:], Etok[:])
                nc.vector.max_index(mi8[:], mx8[:], Etok[:])
                nc.vector.tensor_copy(sff[:], mi8[:, 0:1])
                nc.vector.tensor_scalar(dff[:], sff[:], -1.0, 80.0,
                                        Op.mult, Op.add)
                # --- yhat barrel gather: off = s*, width 160 ---
                nc.vector.tensor_copy(offI[:], mi8[:, 0:1])
                mkbits(offI)
                barrel(yap, wb3, 160, 80)
                nc.scalar.copy(yhath[:], wb3[:, 0:160])
                # --- H = W_enc @ yhat^T + b_enc (h-major, bias folded) ---
                yh0p = pp.tile([128, 128], BF16, tag="tp")
                nc.tensor.transpose(yh0p[:], yhath[:, 0:128], id_t[:])
                nc.scalar.copy(yh0[:], yh0p[:])
                yh1p = pp.tile([128, 128], BF16, tag="tp")
                nc.tensor.transpose(yh1p[0:32, :], yhath[:, 128:160], id_t[:])
                nc.scalar.copy(yh1[0:32, :], yh1p[0:32, :])
                Hps = pph.tile([128, 512], mybir.dt.float32, tag="Hps")
                for hc in range(4):
                    r = slice(hc * 128, (hc + 1) * 128)
                    nc.tensor.matmul(Hps[:, r], We1_t[:, r], yh0[:],
                                     start=True, stop=False)
                    nc.tensor.matmul(Hps[:, r], We2_t[:, r], yh1[:],
                                     start=False, stop=True)
                nc.scalar.copy(hsb[:], Hps[:])
                # --- X = W_src @ h + b_src (token-major via flip) ---
                Xps = ppx.tile([128, 160], mybir.dt.float32, tag="Xps")
                for hc in range(4):
                    nc.tensor.matmul(Xps[:], hsb[:, hc * 128:(hc + 1) * 128],
                                     Ws_t[:, hc * 160:(hc + 1) * 160],
                                     start=(hc == 0), stop=False)
                nc.tensor.matmul(Xps[:], on_t[:], bs_t[:], start=False,
                                 stop=True)
                nc.scalar.copy(xext[:], Xps[:])
                # --- y_ele barrel gather: off = d* = 80 - s* ---
                nc.vector.tensor_copy(offI[:], dff[:])
                mkbits(offI)
                barrel(xext, wb2, 80, 80)
                yele = wb2
                # --- state update first (unblocks next iter's dot), then
                # loss from the identity  y_ele - y_res_old = -y_res_new ---
                nc.vector.tensor_tensor(yres[:], yres[:], yele[:, 0:80],
                                        Op.subtract)
                nc.vector.tensor_tensor(dtmp[:], yres[:], keep[:], Op.mult)
                nc.scalar.activation(dsq[:], dtmp[:], AF.Square,
                                     accum_out=lossp[:, it:it + 1])

            nc.sync.dma_start(d_out[:], lossp[:])
    return nc


def kernel(x, y, W_enc, b_enc, W_src, b_src):
    import sys
    if '/opt/trn_rl_repo' not in sys.path:
        sys.path.insert(0, '/opt/trn_rl_repo')
    x = np.asarray(x, np.float32)
    y = np.asarray(y, np.float32)
    consts = _build_consts(W_enc, b_enc, W_src, b_src)

    if "nc" not in _cache:
        _cache["macscan"] = _get_macscan()
        _cache["nc"] = _build_nc()
        _cache["nc"].finalize()
    nc = _cache["nc"]

    xt = x.reshape(NTOK, IDIM)
    yt = y.reshape(NTOK, ODIM)
    in_maps = []
    for c in range(NCORES):
        m = dict(consts)
        m["x"] = np.ascontiguousarray(xt[c * P:(c + 1) * P])
        m["y"] = np.ascontiguousarray(yt[c * P:(c + 1) * P])
        in_maps.append(m)

    from concourse.bass_utils import run_bass_kernel_spmd
    res = run_bass_kernel_spmd(nc, in_maps, list(range(NCORES)))
    parts = np.stack([r["losspart"] for r in res.results])
    keep_cnt = max(int((y != 0.0).sum()), 1)
    nums = parts[:, :, :THINK_ITER].sum(axis=(0, 1), dtype=np.float64)
    losses = (nums / keep_cnt).astype(np.float32)
    return np.float32(np.mean(losses))


# revision 61
# speedup vs baseline: 1.1194x; 1.1194x over previous
"""Trainium2 Bass kernel for nn_Net_17532056502451.

5 "think" iterations: shift-window cosine selector (159 shifts) + softmax
attention + scatter-back + conv-style encoder/decoder with energy argmax
(81 shifts), masked-MSE losses averaged.  Data-parallel: 1024 tokens over
8 cores, 128 tokens/core (one per SBUF partition), token-major.

v2 mappings per core:
- dot correlation: ONE bf16 tensor_tensor (2 elem/cyc) building all 12720
  products + bf16 tree adds (80->40->20->10->5) + fp32 tensor_reduce.
- sliding norms: Square + prefix-scan + strided diff (fp32).
- per-token dynamic windows: GPSIMD ap_gather (idx[p,j] = off_p + j, 16
  candidate lanes) + mask-mult + tensor_reduce diagonal extract (2 DVE ops
  instead of 16 predicated copies).
- energy: quadratic Gram form, all-bf16 PE pipeline: z built in one DVE op,
  transposed on PE (bf16, 1 cyc/row), PSUM->SBUF copies batched 4 chunks at
  a time, E matmuls flipped (z chunk stationary, Az moving) so E lands
  token-major in PSUM - no transpose-back.
- encoder: h-major H matmuls with b_enc folded in via a ones row;
  decoder: X matmuls flipped token-major with b_src via a k=1 matmul -
  no back-transposes.
- softmax exp + denominator in one scalar-engine op (accum_out); loss
  sum-of-squares via scalar Square + accum_out.
"""
import numpy as np

IDIM = 80
ODIM = 80
HDIM = 512
THINK_ITER = 5
TEMPER = 0.7
B, T = 4, 256
NTOK = B * T
P = 128
NCORES = 8
S1 = 159
S2 = 81
NFEAT = 80 * 80
NCHUNK = NFEAT // 128   # 50

_cache = {}


def _bf16(a):
    import ml_dtypes
    return np.asarray(a, dtype=ml_dtypes.bfloat16)


def _register_ops():
    """Custom DVE ops, registered at runtime (documented extension point:
    the per-NEFF DVE table).

    MACSCAN_ANT: out[p,k] = running sum of in0[p,:k+1]*in1[p,:k+1] — one
    full-rate pass computes all windowed dot products; window sums are
    recovered by differencing page-end prefix values.
    SIGNSQ_ANT: out = in0*|in0|*in1 — the selector score in one op.
    """
    import concourse.dve_ops as dve_ops
    from concourse.dve_ops import DveOp, OPS, DveOpSpec
    from concourse.dve_spec import Spec, Src0, Src1, Zero, scan, lower, relu
    from concourse.dve_spec import AluOp as SAlu
    from concourse.dve_spec import _has_src1 as has_src1

    have = {o.name: o for o in OPS}
    if "MACSCAN_ANT" in have and "SIGNSQ_ANT" in have:
        return have["MACSCAN_ANT"], have["SIGNSQ_ANT"]

    def ref_scan(in0, in1):
        p = in0.shape[0]
        prod = (np.asarray(in0, np.float64) * np.asarray(in1, np.float64))
        return np.cumsum(prod.reshape(p, -1), axis=1).reshape(
            in0.shape).astype(np.float32)

    def ref_signsq(in0, in1):
        return in0 * np.abs(in0) * in1

    a = relu(Src0) + relu(Zero - Src0)
    specs = [("MACSCAN_ANT", Spec(body=scan(SAlu.ADD, Src0 * Src1),
                                  reference=ref_scan)),
             ("SIGNSQ_ANT", Spec(body=Src0 * a * Src1, reference=ref_signsq))]
    out = []
    for name, sp in specs:
        row = max(dve_ops._SUB_OPCODE_FOR_NAME.values()) + 1
        dve_ops._SUB_OPCODE_FOR_NAME[name] = row
        shas = {}
        for ver in ("v3", "v4"):
            tmp = DveOpSpec(name=name, opcode=row, uops=lower(sp, ver=ver),
                            rd1_en=has_src1(sp))
            shas[ver] = tmp.sha(ver)
        op = DveOp(name, sp, subdim=False, uops_sha=shas)
        OPS.append(op)
        dve_ops.CUSTOM_DVE_SPECS[op.name] = op.spec
        out.append(op)
    return out[0], out[1]


def _build_consts(W_enc, b_enc, W_src, b_src):
    W_enc = np.asarray(W_enc, np.float32)
    b_enc = np.asarray(b_enc, np.float32)
    W_src = np.asarray(W_src, np.float32)
    b_src = np.asarray(b_src, np.float32)
    C = (W_enc.T @ W_enc).astype(np.float32)
    q = (W_enc.T @ b_enc).astype(np.float32)
    bb = np.float32(b_enc @ b_enc)
    # E[t,s] = sum_{d,i} Az[s, 80d+i] * ye_i ye_{i+d} + sum_i 2 q[dd+i] ye_i
    #          + bb,  dd = 80 - s
    Az = np.zeros((S2, NFEAT), np.float32)
    Al = np.zeros((S2, 80), np.float32)
    for s in range(S2):
        dd = 80 - s
        blk = C[dd:dd + 80, dd:dd + 80]
        for d in range(80):
            diag = np.diagonal(blk, offset=d).copy()
            Az[s, d * 80: d * 80 + (80 - d)] = (2.0 if d > 0 else 1.0) * diag
        Al[s, :80] = 2.0 * q[dd:dd + 80]
        # bb (constant across s and t) dropped: argmax-invariant shift
    Az_cat = np.ascontiguousarray(Az.T)               # (6400, 81): pi-major
    Atail = np.ascontiguousarray(Al.T)                # (80, 81)
    W_encT = np.ascontiguousarray(W_enc.T)            # (160, 512)
    We1 = W_encT[0:128]                               # (128, 512)
    We2 = np.zeros((33, 512), np.float32)             # 32 c's + bias row
    We2[0:32] = W_encT[128:160]
    We2[32] = b_enc
    W_srcT = np.ascontiguousarray(W_src.T)            # (512, 160)
    iota8 = np.broadcast_to(np.arange(8, dtype=np.int32), (P, 8)).copy()
    ident = np.eye(128, dtype=np.float32)
    return dict(Az=_bf16(Az_cat), Atail=_bf16(Atail),
                We1=_bf16(We1), We2=_bf16(We2), Ws=_bf16(W_srcT),
                bsrc=_bf16(b_src.reshape(1, 160)),
                ones1=_bf16(np.ones((1, 128), np.float32)),
                ident=_bf16(ident), iota8=iota8)


def _build_nc():
    import concourse.bass as bass
    import concourse.bacc as bacc
    import concourse.mybir as mybir
    from concourse.tile import TileContext

    F32 = mybir.dt.float32
    BF16 = mybir.dt.bfloat16
    I16 = mybir.dt.int16
    U32 = mybir.dt.uint32
    Op = mybir.AluOpType
    AF = mybir.ActivationFunctionType
    AX = mybir.AxisListType

    nc = bacc.Bacc()
    d_x = nc.declare_dram_parameter("x", [P, 80], F32, isOutput=False)
    d_y = nc.declare_dram_parameter("y", [P, 80], F32, isOutput=False)
    d_A = nc.declare_dram_parameter("Az", [NFEAT, 81], BF16, isOutput=False)
    d_At = nc.declare_dram_parameter("Atail", [80, 81], BF16, isOutput=False)
    d_We1 = nc.declare_dram_parameter("We1", [128, 512], BF16, isOutput=False)
    d_We2 = nc.declare_dram_parameter("We2", [33, 512], BF16, isOutput=False)
    d_Ws = nc.declare_dram_parameter("Ws", [512, 160], BF16, isOutput=False)
    d_bs = nc.declare_dram_parameter("bsrc", [1, 160], BF16, isOutput=False)
    d_on = nc.declare_dram_parameter("ones1", [1, 128], BF16, isOutput=False)
    d_id = nc.declare_dram_parameter("ident", [128, 128], BF16, isOutput=False)
    d_i8 = nc.declare_dram_parameter("iota8", [P, 8], mybir.dt.int32,
                                     isOutput=False)
    d_out = nc.declare_dram_parameter("losspart", [P, 8], F32, isOutput=True)

    with TileContext(nc) as tc:
        with (
            tc.tile_pool(name="const", bufs=1) as cpool,
            tc.tile_pool(name="work", bufs=1) as pool,
            tc.tile_pool(name="ps_rot", bufs=2, space="PSUM") as pp,
            tc.tile_pool(name="ps_e", bufs=1, space="PSUM") as ppe,
            tc.tile_pool(name="ps_h", bufs=1, space="PSUM") as pph,
            tc.tile_pool(name="ps_x", bufs=1, space="PSUM") as ppx,
        ):
            # ---- inputs first: the first iteration only needs x/y ----
            xpad = pool.tile([P, 238], F32, tag="xpad")
            yres = pool.tile([P, 80], F32, tag="yres")
            nc.vector.memset(xpad[:], 0.0)
            nc.sync.dma_start(xpad[:, 79:159], d_x[:])
            nc.sync.dma_start(yres[:], d_y[:])
            # ---- constants ----
            A_t = cpool.tile([P, NCHUNK * 81], BF16, tag="A")
            for k in range(NCHUNK):
                nc.sync.dma_start(A_t[:, k * 81:(k + 1) * 81],
                                  d_A[k * 128:(k + 1) * 128, :])
            At_t = cpool.tile([80, 81], BF16, tag="At")
            nc.sync.dma_start(At_t[:], d_At[:])
            We1_t = cpool.tile([128, 512], BF16, tag="We1")
            nc.sync.dma_start(We1_t[:], d_We1[:])
            We2_t = cpool.tile([33, 512], BF16, tag="We2")
            nc.sync.dma_start(We2_t[:], d_We2[:])
            Ws_t = cpool.tile([P, 4 * 160], BF16, tag="Ws")
            for k in range(4):
                nc.sync.dma_start(Ws_t[:, k * 160:(k + 1) * 160],
                                  d_Ws[k * 128:(k + 1) * 128, :])
            bs_t = cpool.tile([1, 160], BF16, tag="bs")
            nc.sync.dma_start(bs_t[:], d_bs[:])
            on_t = cpool.tile([1, 128], BF16, tag="on")
            nc.sync.dma_start(on_t[:], d_on[:])
            id_t = cpool.tile([128, 128], BF16, tag="id")
            nc.sync.dma_start(id_t[:], d_id[:])
            i8_t = cpool.tile([P, 8], mybir.dt.int32, tag="i8")
            nc.sync.dma_start(i8_t[:], d_i8[:])

            # ---- state ----
            keep = pool.tile([P, 80], F32, tag="keep")
            yap = pool.tile([P, 240], F32, tag="yap")
            lossp = pool.tile([P, 8], F32, tag="lossp")
            yah = pool.tile([P, 160], BF16, tag="yah")
            nc.vector.memset(yap[:], 0.0)
            nc.vector.memset(lossp[:], 0.0)
            nc.vector.tensor_scalar(keep[:], yres[:], 0.0, None, Op.not_equal)

            # dot workspace
            dscan = pool.tile([P, S1 * 80], F32, tag="dscan")
            dot = pool.tile([P, S1], F32, tag="dot")
            gsel = pool.tile([P, S1], F32, tag="gsel")
            cs2 = pool.tile([P, 238], F32, tag="cs2")
            nsq = pool.tile([P, S1], F32, tag="nsq")
            rnsq = pool.tile([P, S1], F32, tag="rnsq")

            I32 = mybir.dt.int32
            mx8 = pool.tile([P, 8], F32, tag="mx8")
            mi8 = pool.tile([P, 8], U32, tag="mi8")
            thf = pool.tile([P, 1], F32, tag="thf")
            thxf = pool.tile([P, 1], F32, tag="thxf")
            sff = pool.tile([P, 1], F32, tag="sff")
            dff = pool.tile([P, 1], F32, tag="dff")
            offI = pool.tile([P, 1], I32, tag="offI")
            bits = pool.tile([P, 8], I32, tag="bits")
            onesI = pool.tile([P, 8], I32, tag="onesI")
            nc.vector.memset(onesI[:], 1)
            wb1 = pool.tile([P, 240], F32, tag="wb1")
            wb2 = pool.tile([P, 240], F32, tag="wb2")
            wb3 = pool.tile([P, 240], F32, tag="wb3")
            yhath = pool.tile([P, 160], BF16, tag="yhath")
            zt = pool.tile([P, 80], F32, tag="zt")
            et = pool.tile([P, 80], F32, tag="et")
            ssum = pool.tile([P, 1], F32, tag="ssum")
            rsum = pool.tile([P, 1], F32, tag="rsum")
            nzm = pool.tile([P, 1], F32, tag="nzm")

            zfeat = pool.tile([P, NFEAT], BF16, tag="zfeat")
            zsb = pool.tile([P, NFEAT], BF16, tag="zsb")
            etl = pool.tile([128, 128], BF16, tag="etl")
            yh0 = pool.tile([128, 128], BF16, tag="yh0")
            yh1 = pool.tile([33, 128], BF16, tag="yh1")
            nc.sync.dma_start(yh1[32:33, :], d_on[:])
            hsb = pool.tile([128, 512], BF16, tag="hsb")
            dtmp = pool.tile([P, 80], F32, tag="dtmp")
            dsq = pool.tile([P, 80], F32, tag="dsq")

            macscan = _cache["macscan"]
            signsq = _cache["signsq"]

            def sliding_norms():
                # cs2[j] = sum_{m<=j} xpad[m]^2; nsq[s] = cs2[s+79]-cs2[s-1];
                # rnsq = 1/max(nsq, eps) precomputed off the critical path
                nc.vector._custom_dve(macscan, out=cs2[:], in0=xpad[:],
                                      in1=xpad[:])
                nc.vector.tensor_tensor(nsq[:, 1:159], cs2[:, 80:238],
                                        cs2[:, 0:158], Op.subtract)
                nc.vector.tensor_copy(nsq[:, 0:1], cs2[:, 79:80])
                nc.vector.tensor_scalar_max(rnsq[:], nsq[:], 1e-30)
                nc.vector.reciprocal(rnsq[:], rnsq[:])
                # prefetch the y_align barrel source
                nc.vector.tensor_copy(wb1[:, 0:238], xpad[:])

            sliding_norms()

            def barrel(src, wb, out_w, maxoff):
                """wb[p, j] <- src[p, off_p + j] for j in [0, out_w).

                bits must already hold (off >> b) & 1.  In-place left shifts:
                per-partition masks mean untouched rows keep their values;
                shifted reads are ahead of writes (stream order) so in-place
                is safe.  src=None means wb was pre-loaded by the caller.
                """
                nbits = (maxoff).bit_length()
                cw = out_w + maxoff
                if src is not None:
                    nc.vector.tensor_copy(wb[:, 0:cw], src[:, 0:cw])
                for b in range(nbits - 1, -1, -1):
                    sh = 1 << b
                    w = out_w + min(maxoff, 2 * sh - 1) - sh
                    nc.vector.copy_predicated(
                        wb[:, 0:w], bits[:, b:b + 1].to_broadcast((P, w)),
                        wb[:, sh:sh + w])

            def mkbits(src_i32):
                nc.vector.tensor_tensor(
                    bits[:], src_i32[:, 0:1].to_broadcast((P, 8)), i8_t[:],
                    Op.logical_shift_right)
                nc.vector.tensor_tensor(bits[:], bits[:], onesI[:],
                                        Op.bitwise_and)

            for it in range(THINK_ITER):
                # --- dot[p,s] = sum_c xpad[p,s+c]*yres[p,c]: one MAC-scan,
                # windowed sums recovered by differencing page ends ---
                in0 = bass.AP(xpad.tensor, xpad.offset,
                              [list(xpad.ap[0]), [1, S1], [1, 80]])
                in1 = bass.AP(yres.tensor, yres.offset,
                              [list(yres.ap[0]), [0, S1], [1, 80]])
                dview = dscan[:].rearrange("p (s c) -> p s c", c=80)
                nc.vector._custom_dve(macscan, out=dview, in0=in0, in1=in1)
                pe0 = bass.AP(dscan.tensor, dscan.offset + 79, [list(dscan.ap[0]), [80, 158]])
                pe1 = bass.AP(dscan.tensor, dscan.offset + 159, [list(dscan.ap[0]), [80, 158]])
                nc.vector.tensor_tensor(dot[:, 1:159], pe1, pe0, Op.subtract)
                nc.vector.tensor_copy(dot[:, 0:1], dscan[:, 79:80])
                # --- theta = argmax dot*|dot|/nsq (fused custom op) ---
                nc.vector._custom_dve(signsq, out=gsel[:], in0=dot[:],
                                      in1=rnsq[:])
                nc.vector.max(mx8[:], gsel[:])
                nc.vector.max_index(mi8[:], mx8[:], gsel[:])
                nc.vector.tensor_copy(thf[:], mi8[:, 0:1])
                # --- y_align barrel gather: yal = wb1[:, 0:80] ---
                nc.vector.tensor_copy(offI[:], mi8[:, 0:1])
                mkbits(offI)
                barrel(None, wb1, 80, 158)
                yal = wb1
                # --- softmax attention -> y_att in yap[:, 80:160] ---
                nc.vector.tensor_tensor(zt[:], yal[:, 0:80], yres[:], Op.mult)
                nc.vector.max(mx8[:], zt[:])
                nc.vector.tensor_scalar_mul(nzm[:], mx8[:, 0:1], -1.0 / TEMPER)
                nc.scalar.activation(et[:], zt[:], AF.Exp, bias=nzm[:, 0:1],
                                     scale=1.0 / TEMPER,
                                     accum_out=ssum[:, 0:1])
                nc.vector.reciprocal(rsum[:], ssum[:])
                nc.vector.scalar_tensor_tensor(yap[:, 80:160], et[:],
                                               rsum[:, 0:1], yal[:, 0:80],
                                               Op.mult, Op.mult)
                nc.scalar.copy(yah[:], yap[:, 80:240])
                # --- tail features: ya^T via DMA xbar transpose ---
                nc.sync.dma_start_transpose(etl[:], yah[:, 0:128])
                # --- z features z[p, 80d+i] = ye[i]*ye[i+d], built and
                # xbar-transposed in halves so DVE/DMA/PE pipeline ---
                Eps = ppe.tile([P, S2], mybir.dt.float32, tag="Eps")
                half = NCHUNK // 2
                for h in range(2):
                    k0, k1 = h * half, (h + 1) * half
                    zin0 = bass.AP(yah.tensor, yah.offset,
                                   [list(yah.ap[0]), [0, 40], [1, 80]])
                    zin1 = bass.AP(yah.tensor, yah.offset + 40 * h,
                                   [list(yah.ap[0]), [1, 40], [1, 80]])
                    zv = zfeat[:, k0 * 128:k1 * 128].rearrange(
                        "p (d i) -> p d i", i=80)
                    nc.vector.tensor_tensor(zv, zin0, zin1, Op.mult)
                    zo = zsb[:, k0 * 128:k1 * 128].rearrange(
                        "p (k t) -> p k t", t=128)
                    nc.sync.dma_start_transpose(
                        zo, zfeat[:, k0 * 128:k1 * 128])
                for h in range(2):
                    k0, k1 = h * half, (h + 1) * half
                    for k in range(k0, k1):
                        nc.tensor.matmul(
                            Eps[:], zsb[:, k * 128:(k + 1) * 128],
                            A_t[:, k * 81:(k + 1) * 81],
                            start=(k == 0), stop=False)
                nc.tensor.matmul(Eps[:], etl[0:80, :], At_t[:], start=False,
                                 stop=True)
                # --- x_ele gather (overlaps E pipe): off = 159 - theta ---
                nc.vector.tensor_scalar(thxf[:], thf[:], -1.0, 159.0,
                                        Op.mult, Op.add)
                nc.vector.tensor_copy(offI[:], thxf[:])
                mkbits(offI)
                barrel(yap, wb2, 80, 159)
                nc.vector.tensor_tensor(xpad[:, 79:159], xpad[:, 79:159],
                                        wb2[:, 0:80], Op.subtract)
                # next-iter sliding norms (overlaps E pipe)
                sliding_norms()
                # --- s* = argmax E (straight from PSUM), d* = 80-s* ---
                nc.vector.max(mx8[:], Eps[:])
                nc.vector.max_index(mi8[:], mx8[:], Eps[:])
                nc.vector.tensor_copy(sff[:], mi8[:, 0:1])
                nc.vector.tensor_scalar(dff[:], sff[:], -1.0, 80.0,
                                        Op.mult, Op.add)
                # --- yhat barrel gather: off = s*, width 160 ---
                nc.vector.tensor_copy(offI[:], mi8[:, 0:1])
                mkbits(offI)
                barrel(yap, wb3, 160, 80)
                nc.scalar.copy(yhath[:], wb3[:, 0:160])
                # --- H = W_enc @ yhat^T + b_enc (h-major, bias folded) ---
                yh0p = pp.tile([128, 128], BF16, tag="tp")
                nc.tensor.transpose(yh0p[:], yhath[:, 0:128], id_t[:])
                nc.scalar.copy(yh0[:], yh0p[:])
                yh1p = pp.tile([128, 128], BF16, tag="tp")
                nc.tensor.transpose(yh1p[0:32, :], yhath[:, 128:160], id_t[:])
                nc.scalar.copy(yh1[0:32, :], yh1p[0:32, :])
                Hps = pph.tile([128, 512], mybir.dt.float32, tag="Hps")
                for hc in range(4):
                    r = slice(hc * 128, (hc + 1) * 128)
                    nc.tensor.matmul(Hps[:, r], We1_t[:, r], yh0[:],
                                     start=True, stop=False)
                    nc.tensor.matmul(Hps[:, r], We2_t[:, r], yh1[:],
                                     start=False, stop=True)
                nc.scalar.copy(hsb[:], Hps[:])
                # --- X = W_src @ h + b_src (token-major via flip) ---
                Xps = ppx.tile([128, 160], mybir.dt.float32, tag="Xps")
                for hc in range(4):
                    nc.tensor.matmul(Xps[:], hsb[:, hc * 128:(hc + 1) * 128],
                                     Ws_t[:, hc * 160:(hc + 1) * 160],
                                     start=(hc == 0), stop=False)
                nc.tensor.matmul(Xps[:], on_t[:], bs_t[:], start=False,
                                 stop=True)
                # --- y_ele barrel gather straight from PSUM: off = d* ---
                nc.vector.tensor_copy(offI[:], dff[:])
                mkbits(offI)
                barrel(Xps, wb2, 80, 80)
                yele = wb2
                # --- state update first (unblocks next iter's dot), then
                # loss from the identity  y_ele - y_res_old = -y_res_new ---
                nc.vector.tensor_tensor(yres[:], yres[:], yele[:, 0:80],
                                        Op.subtract)
                nc.vector.tensor_tensor(dtmp[:], yres[:], keep[:], Op.mult)
                nc.scalar.activation(dsq[:], dtmp[:], AF.Square,
                                     accum_out=lossp[:, it:it + 1])

            nc.sync.dma_start(d_out[:], lossp[:])
    return nc


def kernel(x, y, W_enc, b_enc, W_src, b_src):
    import sys
    if '/opt/trn_rl_repo' not in sys.path:
        sys.path.insert(0, '/opt/trn_rl_repo')
    x = np.asarray(x, np.float32)
    y = np.asarray(y, np.float32)
    consts = _build_consts(W_enc, b_enc, W_src, b_src)

    if "nc" not in _cache:
        _cache["macscan"], _cache["signsq"] = _register_ops()
        _cache["nc"] = _build_nc()
        _cache["nc"].finalize()
    nc = _cache["nc"]

    xt = x.reshape(NTOK, IDIM)
    yt = y.reshape(NTOK, ODIM)
    in_maps = []
    for c in range(NCORES):
        m = dict(consts)
        m["x"] = np.ascontiguousarray(xt[c * P:(c + 1) * P])
        m["y"] = np.ascontiguousarray(yt[c * P:(c + 1) * P])
        in_maps.append(m)

    from concourse.bass_utils import run_bass_kernel_spmd
    res = run_bass_kernel_spmd(nc, in_maps, list(range(NCORES)))
    parts = np.stack([r["losspart"] for r in res.results])
    keep_cnt = max(int((y != 0.0).sum()), 1)
    nums = parts[:, :, :THINK_ITER].sum(axis=(0, 1), dtype=np.float64)
    losses = (nums / keep_cnt).astype(np.float32)
    return np.float32(np.mean(losses))


# revision 64
# speedup vs baseline: 1.1242x; 1.0043x over previous
"""Trainium2 Bass kernel for nn_Net_17532056502451.

5 "think" iterations: shift-window cosine selector (159 shifts) + softmax
attention + scatter-back + conv-style encoder/decoder with energy argmax
(81 shifts), masked-MSE losses averaged.  Data-parallel: 1024 tokens over
8 cores, 128 tokens/core (one per SBUF partition), token-major.

v2 mappings per core:
- dot correlation: ONE bf16 tensor_tensor (2 elem/cyc) building all 12720
  products + bf16 tree adds (80->40->20->10->5) + fp32 tensor_reduce.
- sliding norms: Square + prefix-scan + strided diff (fp32).
- per-token dynamic windows: GPSIMD ap_gather (idx[p,j] = off_p + j, 16
  candidate lanes) + mask-mult + tensor_reduce diagonal extract (2 DVE ops
  instead of 16 predicated copies).
- energy: quadratic Gram form, all-bf16 PE pipeline: z built in one DVE op,
  transposed on PE (bf16, 1 cyc/row), PSUM->SBUF copies batched 4 chunks at
  a time, E matmuls flipped (z chunk stationary, Az moving) so E lands
  token-major in PSUM - no transpose-back.
- encoder: h-major H matmuls with b_enc folded in via a ones row;
  decoder: X matmuls flipped token-major with b_src via a k=1 matmul -
  no back-transposes.
- softmax exp + denominator in one scalar-engine op (accum_out); loss
  sum-of-squares via scalar Square + accum_out.
"""
import numpy as np

IDIM = 80
ODIM = 80
HDIM = 512
THINK_ITER = 5
TEMPER = 0.7
B, T = 4, 256
NTOK = B * T
P = 128
NCORES = 8
S1 = 159
S2 = 81
NFEAT = 80 * 80
NCHUNK = NFEAT // 128   # 50

_cache = {}


def _bf16(a):
    import ml_dtypes
    return np.asarray(a, dtype=ml_dtypes.bfloat16)


def _register_ops():
    """Custom DVE ops, registered at runtime (documented extension point:
    the per-NEFF DVE table).

    MACSCAN_ANT: out[p,k] = running sum of in0[p,:k+1]*in1[p,:k+1] — one
    full-rate pass computes all windowed dot products; window sums are
    recovered by differencing page-end prefix values.
    SIGNSQ_ANT: out = in0*|in0|*in1 — the selector score in one op.
    """
    import concourse.dve_ops as dve_ops
    from concourse.dve_ops import DveOp, OPS, DveOpSpec
    from concourse.dve_spec import Spec, Src0, Src1, Zero, scan, lower, relu
    from concourse.dve_spec import AluOp as SAlu
    from concourse.dve_spec import _has_src1 as has_src1

    have = {o.name: o for o in OPS}
    if "MACSCAN_ANT" in have and "SIGNSQ_ANT" in have:
        return have["MACSCAN_ANT"], have["SIGNSQ_ANT"]

    def ref_scan(in0, in1):
        p = in0.shape[0]
        prod = (np.asarray(in0, np.float64) * np.asarray(in1, np.float64))
        return np.cumsum(prod.reshape(p, -1), axis=1).reshape(
            in0.shape).astype(np.float32)

    def ref_signsq(in0, in1):
        return in0 * np.abs(in0) * in1

    a = relu(Src0) + relu(Zero - Src0)
    specs = [("MACSCAN_ANT", Spec(body=scan(SAlu.ADD, Src0 * Src1),
                                  reference=ref_scan)),
             ("SIGNSQ_ANT", Spec(body=Src0 * a * Src1, reference=ref_signsq))]
    out = []
    for name, sp in specs:
        row = max(dve_ops._SUB_OPCODE_FOR_NAME.values()) + 1
        dve_ops._SUB_OPCODE_FOR_NAME[name] = row
        shas = {}
        for ver in ("v3", "v4"):
            tmp = DveOpSpec(name=name, opcode=row, uops=lower(sp, ver=ver),
                            rd1_en=has_src1(sp))
            shas[ver] = tmp.sha(ver)
        op = DveOp(name, sp, subdim=False, uops_sha=shas)
        OPS.append(op)
        dve_ops.CUSTOM_DVE_SPECS[op.name] = op.spec
        out.append(op)
    return out[0], out[1]


def _build_consts(W_enc, b_enc, W_src, b_src):
    W_enc = np.asarray(W_enc, np.float32)
    b_enc = np.asarray(b_enc, np.float32)
    W_src = np.asarray(W_src, np.float32)
    b_src = np.asarray(b_src, np.float32)
    C = (W_enc.T @ W_enc).astype(np.float32)
    q = (W_enc.T @ b_enc).astype(np.float32)
    bb = np.float32(b_enc @ b_enc)
    # E[t,s] = sum_{d,i} Az[s, 80d+i] * ye_i ye_{i+d} + sum_i 2 q[dd+i] ye_i
    #          + bb,  dd = 80 - s
    Az = np.zeros((S2, NFEAT), np.float32)
    Al = np.zeros((S2, 80), np.float32)
    for s in range(S2):
        dd = 80 - s
        blk = C[dd:dd + 80, dd:dd + 80]
        for d in range(80):
            diag = np.diagonal(blk, offset=d).copy()
            Az[s, d * 80: d * 80 + (80 - d)] = (2.0 if d > 0 else 1.0) * diag
        Al[s, :80] = 2.0 * q[dd:dd + 80]
        # bb (constant across s and t) dropped: argmax-invariant shift
    Az_cat = np.ascontiguousarray(Az.T)               # (6400, 81): pi-major
    Atail = np.ascontiguousarray(Al.T)                # (80, 81)
    W_encT = np.ascontiguousarray(W_enc.T)            # (160, 512)
    We1 = W_encT[0:128]                               # (128, 512)
    We2 = np.zeros((33, 512), np.float32)             # 32 c's + bias row
    We2[0:32] = W_encT[128:160]
    We2[32] = b_enc
    W_srcT = np.ascontiguousarray(W_src.T)            # (512, 160)
    iota8 = np.broadcast_to(np.arange(8, dtype=np.int32), (P, 8)).copy()
    ident = np.eye(128, dtype=np.float32)
    return dict(Az=_bf16(Az_cat), Atail=_bf16(Atail),
                We1=_bf16(We1), We2=_bf16(We2), Ws=_bf16(W_srcT),
                bsrc=_bf16(b_src.reshape(1, 160)),
                ones1=_bf16(np.ones((1, 128), np.float32)),
                ident=_bf16(ident), iota8=iota8)


def _build_nc():
    import concourse.bass as bass
    import concourse.bacc as bacc
    import concourse.mybir as mybir
    from concourse.tile import TileContext

    F32 = mybir.dt.float32
    BF16 = mybir.dt.bfloat16
    I16 = mybir.dt.int16
    U32 = mybir.dt.uint32
    Op = mybir.AluOpType
    AF = mybir.ActivationFunctionType
    AX = mybir.AxisListType

    nc = bacc.Bacc()
    d_x = nc.declare_dram_parameter("x", [P, 80], F32, isOutput=False)
    d_y = nc.declare_dram_parameter("y", [P, 80], F32, isOutput=False)
    d_A = nc.declare_dram_parameter("Az", [NFEAT, 81], BF16, isOutput=False)
    d_At = nc.declare_dram_parameter("Atail", [80, 81], BF16, isOutput=False)
    d_We1 = nc.declare_dram_parameter("We1", [128, 512], BF16, isOutput=False)
    d_We2 = nc.declare_dram_parameter("We2", [33, 512], BF16, isOutput=False)
    d_Ws = nc.declare_dram_parameter("Ws", [512, 160], BF16, isOutput=False)
    d_bs = nc.declare_dram_parameter("bsrc", [1, 160], BF16, isOutput=False)
    d_on = nc.declare_dram_parameter("ones1", [1, 128], BF16, isOutput=False)
    d_id = nc.declare_dram_parameter("ident", [128, 128], BF16, isOutput=False)
    d_i8 = nc.declare_dram_parameter("iota8", [P, 8], mybir.dt.int32,
                                     isOutput=False)
    d_out = nc.declare_dram_parameter("losspart", [P, 8], F32, isOutput=True)

    with TileContext(nc) as tc:
        with (
            tc.tile_pool(name="const", bufs=1) as cpool,
            tc.tile_pool(name="work", bufs=1) as pool,
            tc.tile_pool(name="ps_rot", bufs=2, space="PSUM") as pp,
            tc.tile_pool(name="ps_e", bufs=1, space="PSUM") as ppe,
            tc.tile_pool(name="ps_h", bufs=1, space="PSUM") as pph,
            tc.tile_pool(name="ps_x", bufs=1, space="PSUM") as ppx,
        ):
            # ---- inputs first: the first iteration only needs x/y ----
            xpad = pool.tile([P, 238], F32, tag="xpad")
            yres = pool.tile([P, 80], F32, tag="yres")
            nc.vector.memset(xpad[:], 0.0)
            nc.sync.dma_start(xpad[:, 79:159], d_x[:])
            nc.sync.dma_start(yres[:], d_y[:])
            # ---- constants ----
            A_t = cpool.tile([P, NCHUNK * 81], BF16, tag="A")
            for k in range(NCHUNK):
                nc.sync.dma_start(A_t[:, k * 81:(k + 1) * 81],
                                  d_A[k * 128:(k + 1) * 128, :])
            At_t = cpool.tile([80, 81], BF16, tag="At")
            nc.sync.dma_start(At_t[:], d_At[:])
            We1_t = cpool.tile([128, 512], BF16, tag="We1")
            nc.sync.dma_start(We1_t[:], d_We1[:])
            We2_t = cpool.tile([33, 512], BF16, tag="We2")
            nc.sync.dma_start(We2_t[:], d_We2[:])
            Ws_t = cpool.tile([P, 4 * 160], BF16, tag="Ws")
            for k in range(4):
                nc.sync.dma_start(Ws_t[:, k * 160:(k + 1) * 160],
                                  d_Ws[k * 128:(k + 1) * 128, :])
            bs_t = cpool.tile([1, 160], BF16, tag="bs")
            nc.sync.dma_start(bs_t[:], d_bs[:])
            on_t = cpool.tile([1, 128], BF16, tag="on")
            nc.sync.dma_start(on_t[:], d_on[:])
            id_t = cpool.tile([128, 128], BF16, tag="id")
            nc.sync.dma_start(id_t[:], d_id[:])
            i8_t = cpool.tile([P, 8], mybir.dt.int32, tag="i8")
            nc.sync.dma_start(i8_t[:], d_i8[:])

            # ---- state ----
            keep = pool.tile([P, 80], F32, tag="keep")
            yap = pool.tile([P, 240], F32, tag="yap")
            lossp = pool.tile([P, 8], F32, tag="lossp")
            yah = pool.tile([P, 160], BF16, tag="yah")
            nc.vector.memset(yap[:], 0.0)
            nc.vector.memset(lossp[:], 0.0)
            nc.vector.tensor_scalar(keep[:], yres[:], 0.0, None, Op.not_equal)

            # dot workspace
            dscan = pool.tile([P, S1 * 80], F32, tag="dscan")
            dot = pool.tile([P, S1], F32, tag="dot")
            gsel = pool.tile([P, S1], F32, tag="gsel")
            cs2 = pool.tile([P, 238], F32, tag="cs2")
            nsq = pool.tile([P, S1], F32, tag="nsq")
            rnsq = pool.tile([P, S1], F32, tag="rnsq")

            I32 = mybir.dt.int32
            mx8 = pool.tile([P, 8], F32, tag="mx8")
            mi8 = pool.tile([P, 8], U32, tag="mi8")
            thf = pool.tile([P, 1], F32, tag="thf")
            thxf = pool.tile([P, 1], F32, tag="thxf")
            sff = pool.tile([P, 1], F32, tag="sff")
            dff = pool.tile([P, 1], F32, tag="dff")
            offI = pool.tile([P, 1], I32, tag="offI")
            bits = pool.tile([P, 8], I32, tag="bits")
            onesI = pool.tile([P, 8], I32, tag="onesI")
            nc.vector.memset(onesI[:], 1)
            wb1 = pool.tile([P, 240], F32, tag="wb1")
            wb2 = pool.tile([P, 240], F32, tag="wb2")
            wb3 = pool.tile([P, 240], F32, tag="wb3")
            yhath = pool.tile([P, 160], BF16, tag="yhath")
            zt = pool.tile([P, 80], F32, tag="zt")
            et = pool.tile([P, 80], F32, tag="et")
            ssum = pool.tile([P, 1], F32, tag="ssum")
            rsum = pool.tile([P, 1], F32, tag="rsum")
            nzm = pool.tile([P, 1], F32, tag="nzm")

            zfeat = pool.tile([P, NFEAT], BF16, tag="zfeat")
            zsb = pool.tile([P, NFEAT], BF16, tag="zsb")
            etl = pool.tile([128, 128], BF16, tag="etl")
            yh0 = pool.tile([128, 128], BF16, tag="yh0")
            yh1 = pool.tile([33, 128], BF16, tag="yh1")
            nc.sync.dma_start(yh1[32:33, :], d_on[:])
            hsb = pool.tile([128, 512], BF16, tag="hsb")
            dtmp = pool.tile([P, 80], F32, tag="dtmp")
            dsq = pool.tile([P, 80], F32, tag="dsq")

            macscan = _cache["macscan"]
            signsq = _cache["signsq"]

            def sliding_norms():
                # cs2[j] = sum_{m<=j} xpad[m]^2; nsq[s] = cs2[s+79]-cs2[s-1];
                # rnsq = 1/max(nsq, eps) precomputed off the critical path
                nc.vector._custom_dve(macscan, out=cs2[:], in0=xpad[:],
                                      in1=xpad[:])
                nc.vector.tensor_tensor(nsq[:, 1:159], cs2[:, 80:238],
                                        cs2[:, 0:158], Op.subtract)
                nc.vector.tensor_copy(nsq[:, 0:1], cs2[:, 79:80])
                nc.vector.tensor_scalar_max(rnsq[:], nsq[:], 1e-30)
                nc.vector.reciprocal(rnsq[:], rnsq[:])
                # prefetch the y_align barrel source
                nc.vector.tensor_copy(wb1[:, 0:238], xpad[:])

            sliding_norms()

            def barrel(src, wb, out_w, maxoff):
                """wb[p, j] <- src[p, off_p + j] for j in [0, out_w).

                bits must already hold (off >> b) & 1.  In-place left shifts:
                per-partition masks mean untouched rows keep their values;
                shifted reads are ahead of writes (stream order) so in-place
                is safe.  src=None means wb was pre-loaded by the caller.
                """
                nbits = (maxoff).bit_length()
                cw = out_w + maxoff
                if src is not None:
                    nc.vector.tensor_copy(wb[:, 0:cw], src[:, 0:cw])
                for b in range(nbits - 1, -1, -1):
                    sh = 1 << b
                    w = out_w + min(maxoff, 2 * sh - 1) - sh
                    nc.vector.copy_predicated(
                        wb[:, 0:w], bits[:, b:b + 1].to_broadcast((P, w)),
                        wb[:, sh:sh + w])

            def mkbits(src_i32):
                nc.vector.tensor_tensor(
                    bits[:], src_i32[:, 0:1].to_broadcast((P, 8)), i8_t[:],
                    Op.logical_shift_right)
                nc.vector.tensor_tensor(bits[:], bits[:], onesI[:],
                                        Op.bitwise_and)

            for it in range(THINK_ITER):
                # --- dot[p,s] = sum_c xpad[p,s+c]*yres[p,c]: one MAC-scan,
                # windowed sums recovered by differencing page ends ---
                in0 = bass.AP(xpad.tensor, xpad.offset,
                              [list(xpad.ap[0]), [1, S1], [1, 80]])
                in1 = bass.AP(yres.tensor, yres.offset,
                              [list(yres.ap[0]), [0, S1], [1, 80]])
                dview = dscan[:].rearrange("p (s c) -> p s c", c=80)
                nc.vector._custom_dve(macscan, out=dview, in0=in0, in1=in1)
                pe0 = bass.AP(dscan.tensor, dscan.offset + 79, [list(dscan.ap[0]), [80, 158]])
                pe1 = bass.AP(dscan.tensor, dscan.offset + 159, [list(dscan.ap[0]), [80, 158]])
                nc.vector.tensor_tensor(dot[:, 1:159], pe1, pe0, Op.subtract)
                nc.vector.tensor_copy(dot[:, 0:1], dscan[:, 79:80])
                # --- theta = argmax dot*|dot|/nsq (fused custom op) ---
                nc.vector._custom_dve(signsq, out=gsel[:], in0=dot[:],
                                      in1=rnsq[:])
                nc.vector.max(mx8[:], gsel[:])
                nc.vector.max_index(mi8[:], mx8[:], gsel[:])
                nc.vector.tensor_copy(thf[:], mi8[:, 0:1])
                # --- y_align barrel gather: yal = wb1[:, 0:80] ---
                nc.vector.tensor_copy(offI[:], mi8[:, 0:1])
                mkbits(offI)
                barrel(None, wb1, 80, 158)
                yal = wb1
                # --- softmax attention -> y_att in yap[:, 80:160] ---
                # (no max-subtraction: |zt|/T is far inside fp32 exp range)
                nc.vector.tensor_tensor(zt[:], yal[:, 0:80], yres[:], Op.mult)
                nc.scalar.activation(et[:], zt[:], AF.Exp,
                                     scale=1.0 / TEMPER,
                                     accum_out=ssum[:, 0:1])
                nc.vector.reciprocal(rsum[:], ssum[:])
                nc.vector.scalar_tensor_tensor(yap[:, 80:160], et[:],
                                               rsum[:, 0:1], yal[:, 0:80],
                                               Op.mult, Op.mult)
                nc.scalar.copy(yah[:], yap[:, 80:240])
                # --- tail features: ya^T via DMA xbar transpose ---
                nc.scalar.dma_start_transpose(etl[:], yah[:, 0:128])
                # --- z features z[p, 80d+i] = ye[i]*ye[i+d], built and
                # xbar-transposed in halves so DVE/DMA/PE pipeline ---
                Eps = ppe.tile([P, S2], mybir.dt.float32, tag="Eps")
                half = NCHUNK // 2
                for h in range(2):
                    k0, k1 = h * half, (h + 1) * half
                    zin0 = bass.AP(yah.tensor, yah.offset,
                                   [list(yah.ap[0]), [0, 40], [1, 80]])
                    zin1 = bass.AP(yah.tensor, yah.offset + 40 * h,
                                   [list(yah.ap[0]), [1, 40], [1, 80]])
                    zv = zfeat[:, k0 * 128:k1 * 128].rearrange(
                        "p (d i) -> p d i", i=80)
                    nc.vector.tensor_tensor(zv, zin0, zin1, Op.mult)
                    zo = zsb[:, k0 * 128:k1 * 128].rearrange(
                        "p (k t) -> p k t", t=128)
                    eng = nc.sync if h == 0 else nc.scalar
                    eng.dma_start_transpose(
                        zo, zfeat[:, k0 * 128:k1 * 128])
                for h in range(2):
                    k0, k1 = h * half, (h + 1) * half
                    for k in range(k0, k1):
                        nc.tensor.matmul(
                            Eps[:], zsb[:, k * 128:(k + 1) * 128],
                            A_t[:, k * 81:(k + 1) * 81],
                            start=(k == 0), stop=False)
                nc.tensor.matmul(Eps[:], etl[0:80, :], At_t[:], start=False,
                                 stop=True)
                # --- x_ele gather (overlaps E pipe): off = 159 - theta ---
                nc.vector.tensor_scalar(thxf[:], thf[:], -1.0, 159.0,
                                        Op.mult, Op.add)
                nc.vector.tensor_copy(offI[:], thxf[:])
                mkbits(offI)
                barrel(yap, wb2, 80, 159)
                nc.vector.tensor_tensor(xpad[:, 79:159], xpad[:, 79:159],
                                        wb2[:, 0:80], Op.subtract)
                # next-iter sliding norms (overlaps E pipe)
                sliding_norms()
                # --- s* = argmax E (straight from PSUM), d* = 80-s* ---
                nc.vector.max(mx8[:], Eps[:])
                nc.vector.max_index(mi8[:], mx8[:], Eps[:])
                nc.vector.tensor_copy(sff[:], mi8[:, 0:1])
                nc.vector.tensor_scalar(dff[:], sff[:], -1.0, 80.0,
                                        Op.mult, Op.add)
                # --- yhat barrel gather: off = s*, width 160 ---
                nc.vector.tensor_copy(offI[:], mi8[:, 0:1])
                mkbits(offI)
                barrel(yap, wb3, 160, 80)
                nc.scalar.copy(yhath[:], wb3[:, 0:160])
                # --- H = W_enc @ yhat^T + b_enc (h-major, bias folded) ---
                yh0p = pp.tile([128, 128], BF16, tag="tp")
                nc.tensor.transpose(yh0p[:], yhath[:, 0:128], id_t[:])
                nc.scalar.copy(yh0[:], yh0p[:])
                yh1p = pp.tile([128, 128], BF16, tag="tp")
                nc.tensor.transpose(yh1p[0:32, :], yhath[:, 128:160], id_t[:])
                nc.scalar.copy(yh1[0:32, :], yh1p[0:32, :])
                Hps = pph.tile([128, 512], mybir.dt.float32, tag="Hps")
                for hc in range(4):
                    r = slice(hc * 128, (hc + 1) * 128)
                    nc.tensor.matmul(Hps[:, r], We1_t[:, r], yh0[:],
                                     start=True, stop=False)
                    nc.tensor.matmul(Hps[:, r], We2_t[:, r], yh1[:],
                                     start=False, stop=True)
                nc.scalar.copy(hsb[:], Hps[:])
                # --- X = W_src @ h + b_src (token-major via flip) ---
                Xps = ppx.tile([128, 160], mybir.dt.float32, tag="Xps")
                for hc in range(4):
                    nc.tensor.matmul(Xps[:], hsb[:, hc * 128:(hc + 1) * 128],
                                     Ws_t[:, hc * 160:(hc + 1) * 160],
                                     start=(hc == 0), stop=False)
                nc.tensor.matmul(Xps[:], on_t[:], bs_t[:], start=False,
                                 stop=True)
                # --- y_ele barrel gather straight from PSUM: off = d* ---
                nc.vector.tensor_copy(offI[:], dff[:])
                mkbits(offI)
                barrel(Xps, wb2, 80, 80)
                yele = wb2
                # --- state update first (unblocks next iter's dot), then
                # loss from the identity  y_ele - y_res_old = -y_res_new ---
                nc.vector.tensor_tensor(yres[:], yres[:], yele[:, 0:80],
                                        Op.subtract)
                nc.vector.tensor_tensor(dtmp[:], yres[:], keep[:], Op.mult)
                nc.scalar.activation(dsq[:], dtmp[:], AF.Square,
                                     accum_out=lossp[:, it:it + 1])

            nc.sync.dma_start(d_out[:], lossp[:])
    return nc


def kernel(x, y, W_enc, b_enc, W_src, b_src):
    import sys
    if '/opt/trn_rl_repo' not in sys.path:
        sys.path.insert(0, '/opt/trn_rl_repo')
    x = np.asarray(x, np.float32)
    y = np.asarray(y, np.float32)
    consts = _build_consts(W_enc, b_enc, W_src, b_src)

    if "nc" not in _cache:
        _cache["macscan"], _cache["signsq"] = _register_ops()
        _cache["nc"] = _build_nc()
        _cache["nc"].finalize()
    nc = _cache["nc"]

    xt = x.reshape(NTOK, IDIM)
    yt = y.reshape(NTOK, ODIM)
    in_maps = []
    for c in range(NCORES):
        m = dict(consts)
        m["x"] = np.ascontiguousarray(xt[c * P:(c + 1) * P])
        m["y"] = np.ascontiguousarray(yt[c * P:(c + 1) * P])
        in_maps.append(m)

    from concourse.bass_utils import run_bass_kernel_spmd
    res = run_bass_kernel_spmd(nc, in_maps, list(range(NCORES)))
    parts = np.stack([r["losspart"] for r in res.results])
    keep_cnt = max(int((y != 0.0).sum()), 1)
    nums = parts[:, :, :THINK_ITER].sum(axis=(0, 1), dtype=np.float64)
    losses = (nums / keep_cnt).astype(np.float32)
    return np.float32(np.mean(losses))


# revision 72
# speedup vs baseline: 1.1549x; 1.0273x over previous
"""Trainium2 Bass kernel for nn_Net_17532056502451.

5 "think" iterations: shift-window cosine selector (159 shifts) + softmax
attention + scatter-back + conv-style encoder/decoder with energy argmax
(81 shifts), masked-MSE losses averaged.  Data-parallel: 1024 tokens over
8 cores, 128 tokens/core (one per SBUF partition), token-major.

v2 mappings per core:
- dot correlation: ONE bf16 tensor_tensor (2 elem/cyc) building all 12720
  products + bf16 tree adds (80->40->20->10->5) + fp32 tensor_reduce.
- sliding norms: Square + prefix-scan + strided diff (fp32).
- per-token dynamic windows: GPSIMD ap_gather (idx[p,j] = off_p + j, 16
  candidate lanes) + mask-mult + tensor_reduce diagonal extract (2 DVE ops
  instead of 16 predicated copies).
- energy: quadratic Gram form, all-bf16 PE pipeline: z built in one DVE op,
  transposed on PE (bf16, 1 cyc/row), PSUM->SBUF copies batched 4 chunks at
  a time, E matmuls flipped (z chunk stationary, Az moving) so E lands
  token-major in PSUM - no transpose-back.
- encoder: h-major H matmuls with b_enc folded in via a ones row;
  decoder: X matmuls flipped token-major with b_src via a k=1 matmul -
  no back-transposes.
- softmax exp + denominator in one scalar-engine op (accum_out); loss
  sum-of-squares via scalar Square + accum_out.
"""
import numpy as np

IDIM = 80
ODIM = 80
HDIM = 512
THINK_ITER = 5
TEMPER = 0.7
B, T = 4, 256
NTOK = B * T
P = 128
NCORES = 8
S1 = 159
S2 = 81
NFEAT = 80 * 80
NCHUNK = NFEAT // 128   # 50

_cache = {}


def _bf16(a):
    import ml_dtypes
    return np.asarray(a, dtype=ml_dtypes.bfloat16)


def _register_ops():
    """Custom DVE ops, registered at runtime (documented extension point:
    the per-NEFF DVE table).

    MACSCAN_ANT: out[p,k] = running sum of in0[p,:k+1]*in1[p,:k+1] — one
    full-rate pass computes all windowed dot products; window sums are
    recovered by differencing page-end prefix values.
    SIGNSQ_ANT: out = in0*|in0|*in1 — the selector score in one op.
    """
    import concourse.dve_ops as dve_ops
    from concourse.dve_ops import DveOp, OPS, DveOpSpec
    from concourse.dve_spec import Spec, Src0, Src1, Zero, scan, lower, relu
    from concourse.dve_spec import AluOp as SAlu
    from concourse.dve_spec import _has_src1 as has_src1

    have = {o.name: o for o in OPS}
    if "MACSCAN_ANT" in have:
        return have["MACSCAN_ANT"], have["SIGNSQ_ANT"], have["MSQACC_ANT"]

    def ref_scan(in0, in1):
        p = in0.shape[0]
        prod = (np.asarray(in0, np.float64) * np.asarray(in1, np.float64))
        return np.cumsum(prod.reshape(p, -1), axis=1).reshape(
            in0.shape).astype(np.float32)

    def ref_signsq(in0, in1):
        return in0 * np.abs(in0) * in1

    def ref_msq(in0, in1):
        return (in0 * in1) ** 2

    from concourse.dve_spec import sq
    a = relu(Src0) + relu(Zero - Src0)
    specs = [("MACSCAN_ANT", Spec(body=scan(SAlu.ADD, Src0 * Src1),
                                  reference=ref_scan)),
             ("SIGNSQ_ANT", Spec(body=Src0 * a * Src1, reference=ref_signsq)),
             ("MSQACC_ANT", Spec(body=sq(Src0 * Src1), accum=SAlu.ADD,
                                 reference=ref_msq))]
    out = []
    for name, sp in specs:
        row = max(dve_ops._SUB_OPCODE_FOR_NAME.values()) + 1
        dve_ops._SUB_OPCODE_FOR_NAME[name] = row
        shas = {}
        for ver in ("v3", "v4"):
            tmp = DveOpSpec(name=name, opcode=row, uops=lower(sp, ver=ver),
                            rd1_en=has_src1(sp))
            shas[ver] = tmp.sha(ver)
        op = DveOp(name, sp, subdim=False, uops_sha=shas)
        OPS.append(op)
        dve_ops.CUSTOM_DVE_SPECS[op.name] = op.spec
        out.append(op)
    return tuple(out)


def _build_consts(W_enc, b_enc, W_src, b_src):
    W_enc = np.asarray(W_enc, np.float32)
    b_enc = np.asarray(b_enc, np.float32)
    W_src = np.asarray(W_src, np.float32)
    b_src = np.asarray(b_src, np.float32)
    C = (W_enc.T @ W_enc).astype(np.float32)
    q = (W_enc.T @ b_enc).astype(np.float32)
    bb = np.float32(b_enc @ b_enc)
    # E[t,s] = sum_{d,i} Az[s, 80d+i] * ye_i ye_{i+d} + sum_i 2 q[dd+i] ye_i
    #          + bb,  dd = 80 - s
    Az = np.zeros((S2, NFEAT), np.float32)
    Al = np.zeros((S2, 80), np.float32)
    for s in range(S2):
        dd = 80 - s
        blk = C[dd:dd + 80, dd:dd + 80]
        for d in range(80):
            diag = np.diagonal(blk, offset=d).copy()
            Az[s, d * 80: d * 80 + (80 - d)] = (2.0 if d > 0 else 1.0) * diag
        Al[s, :80] = 2.0 * q[dd:dd + 80]
        # bb (constant across s and t) dropped: argmax-invariant shift
    Az_cat = np.ascontiguousarray(Az.T)               # (6400, 81): pi-major
    Atail = np.ascontiguousarray(Al.T)                # (80, 81)
    W_encT = np.ascontiguousarray(W_enc.T)            # (160, 512)
    We1 = W_encT[0:128]                               # (128, 512)
    We2 = np.zeros((33, 512), np.float32)             # 32 c's + bias row
    We2[0:32] = W_encT[128:160]
    We2[32] = b_enc
    W_srcT = np.ascontiguousarray(W_src.T)            # (512, 160)
    iota8 = np.broadcast_to(np.arange(8, dtype=np.int32), (P, 8)).copy()
    ident = np.eye(128, dtype=np.float32)
    return dict(Az=_bf16(Az_cat), Atail=_bf16(Atail),
                We1=_bf16(We1), We2=_bf16(We2), Ws=_bf16(W_srcT),
                bsrc=_bf16(b_src.reshape(1, 160)),
                ones1=_bf16(np.ones((1, 128), np.float32)),
                ident=_bf16(ident), iota8=iota8)


def _build_nc():
    import concourse.bass as bass
    import concourse.bacc as bacc
    import concourse.mybir as mybir
    from concourse.tile import TileContext

    F32 = mybir.dt.float32
    BF16 = mybir.dt.bfloat16
    I16 = mybir.dt.int16
    U32 = mybir.dt.uint32
    Op = mybir.AluOpType
    AF = mybir.ActivationFunctionType
    AX = mybir.AxisListType

    nc = bacc.Bacc()
    d_x = nc.declare_dram_parameter("x", [P, 80], F32, isOutput=False)
    d_y = nc.declare_dram_parameter("y", [P, 80], F32, isOutput=False)
    d_A = nc.declare_dram_parameter("Az", [NFEAT, 81], BF16, isOutput=False)
    d_At = nc.declare_dram_parameter("Atail", [80, 81], BF16, isOutput=False)
    d_We1 = nc.declare_dram_parameter("We1", [128, 512], BF16, isOutput=False)
    d_We2 = nc.declare_dram_parameter("We2", [33, 512], BF16, isOutput=False)
    d_Ws = nc.declare_dram_parameter("Ws", [512, 160], BF16, isOutput=False)
    d_bs = nc.declare_dram_parameter("bsrc", [1, 160], BF16, isOutput=False)
    d_on = nc.declare_dram_parameter("ones1", [1, 128], BF16, isOutput=False)
    d_id = nc.declare_dram_parameter("ident", [128, 128], BF16, isOutput=False)
    d_i8 = nc.declare_dram_parameter("iota8", [P, 8], mybir.dt.int32,
                                     isOutput=False)
    d_out = nc.declare_dram_parameter("losspart", [P, 8], F32, isOutput=True)

    with TileContext(nc) as tc:
        with (
            tc.tile_pool(name="const", bufs=1) as cpool,
            tc.tile_pool(name="work", bufs=1) as pool,
            tc.tile_pool(name="ps_rot", bufs=2, space="PSUM") as pp,
            tc.tile_pool(name="ps_e", bufs=1, space="PSUM") as ppe,
            tc.tile_pool(name="ps_h", bufs=1, space="PSUM") as pph,
            tc.tile_pool(name="ps_x", bufs=1, space="PSUM") as ppx,
        ):
            # ---- inputs first: the first iteration only needs x/y ----
            xpad = pool.tile([P, 238], F32, tag="xpad")
            yres = pool.tile([P, 80], F32, tag="yres")
            nc.vector.memset(xpad[:], 0.0)
            nc.sync.dma_start(xpad[:, 79:159], d_x[:])
            nc.sync.dma_start(yres[:], d_y[:])
            # ---- constants ----
            A_t = cpool.tile([P, NCHUNK * 81], BF16, tag="A")
            for k in range(NCHUNK):
                nc.sync.dma_start(A_t[:, k * 81:(k + 1) * 81],
                                  d_A[k * 128:(k + 1) * 128, :])
            At_t = cpool.tile([80, 81], BF16, tag="At")
            nc.sync.dma_start(At_t[:], d_At[:])
            We1_t = cpool.tile([128, 512], BF16, tag="We1")
            nc.sync.dma_start(We1_t[:], d_We1[:])
            We2_t = cpool.tile([33, 512], BF16, tag="We2")
            nc.sync.dma_start(We2_t[:], d_We2[:])
            Ws_t = cpool.tile([P, 4 * 160], BF16, tag="Ws")
            for k in range(4):
                nc.sync.dma_start(Ws_t[:, k * 160:(k + 1) * 160],
                                  d_Ws[k * 128:(k + 1) * 128, :])
            bs_t = cpool.tile([1, 160], BF16, tag="bs")
            nc.sync.dma_start(bs_t[:], d_bs[:])
            on_t = cpool.tile([1, 128], BF16, tag="on")
            nc.sync.dma_start(on_t[:], d_on[:])
            id_t = cpool.tile([128, 128], BF16, tag="id")
            nc.sync.dma_start(id_t[:], d_id[:])
            i8_t = cpool.tile([P, 8], mybir.dt.int32, tag="i8")
            nc.sync.dma_start(i8_t[:], d_i8[:])

            # ---- state ----
            keep = pool.tile([P, 80], F32, tag="keep")
            yap = pool.tile([P, 240], F32, tag="yap")
            lossp = pool.tile([P, 8], F32, tag="lossp")
            yah = pool.tile([P, 160], BF16, tag="yah")
            nc.vector.memset(yap[:], 0.0)
            nc.vector.memset(lossp[:], 0.0)
            nc.vector.tensor_scalar(keep[:], yres[:], 0.0, None, Op.not_equal)

            # dot workspace
            dscan = pool.tile([P, S1 * 80], F32, tag="dscan")
            dot = pool.tile([P, S1], F32, tag="dot")
            gsel = pool.tile([P, S1], F32, tag="gsel")
            cs2 = pool.tile([P, 238], F32, tag="cs2")
            nsq = pool.tile([P, S1], F32, tag="nsq")
            rnsq = pool.tile([P, S1], F32, tag="rnsq")

            I32 = mybir.dt.int32
            mx8 = pool.tile([P, 8], F32, tag="mx8")
            mi8 = pool.tile([P, 8], U32, tag="mi8")
            thf = pool.tile([P, 1], F32, tag="thf")
            thxf = pool.tile([P, 1], F32, tag="thxf")
            sff = pool.tile([P, 1], F32, tag="sff")
            dff = pool.tile([P, 1], F32, tag="dff")
            offI = pool.tile([P, 1], I32, tag="offI")
            bits = pool.tile([P, 8], I32, tag="bits")
            onesI = pool.tile([P, 8], I32, tag="onesI")
            nc.vector.memset(onesI[:], 1)
            wb1 = pool.tile([P, 240], F32, tag="wb1")
            wb2 = pool.tile([P, 240], F32, tag="wb2")
            wb3 = pool.tile([P, 240], F32, tag="wb3")
            yhath = pool.tile([P, 160], BF16, tag="yhath")
            zt = pool.tile([P, 80], F32, tag="zt")
            et = pool.tile([P, 80], F32, tag="et")
            ssum = pool.tile([P, 1], F32, tag="ssum")
            rsum = pool.tile([P, 1], F32, tag="rsum")
            nzm = pool.tile([P, 1], F32, tag="nzm")

            zfeat = pool.tile([P, NFEAT], BF16, tag="zfeat")
            zsb = pool.tile([P, NFEAT], BF16, tag="zsb")
            etl = pool.tile([128, 128], BF16, tag="etl")
            yh0 = pool.tile([128, 128], BF16, tag="yh0")
            yh1 = pool.tile([33, 128], BF16, tag="yh1")
            nc.sync.dma_start(yh1[32:33, :], d_on[:])
            hsb = pool.tile([128, 512], BF16, tag="hsb")
            dtmp = pool.tile([P, 80], F32, tag="dtmp")
            dsq = pool.tile([P, 80], F32, tag="dsq")

            macscan = _cache["macscan"]
            signsq = _cache["signsq"]
            msqacc = _cache["msqacc"]

            def sliding_norms():
                # cs2[j] = sum_{m<=j} xpad[m]^2; nsq[s] = cs2[s+79]-cs2[s-1];
                # rnsq = 1/max(nsq, eps) precomputed off the critical path
                nc.vector._custom_dve(macscan, out=cs2[:], in0=xpad[:],
                                      in1=xpad[:])
                nc.vector.tensor_tensor(nsq[:, 1:159], cs2[:, 80:238],
                                        cs2[:, 0:158], Op.subtract)
                nc.vector.tensor_copy(nsq[:, 0:1], cs2[:, 79:80])
                nc.vector.tensor_scalar_max(rnsq[:], nsq[:], 1e-30)
                nc.vector.reciprocal(rnsq[:], rnsq[:])
                # prefetch the y_align barrel source
                nc.vector.tensor_copy(wb1[:, 0:238], xpad[:])

            sliding_norms()

            def barrel(src, wb, out_w, maxoff):
                """wb[p, j] <- src[p, off_p + j] for j in [0, out_w).

                bits must already hold (off >> b) & 1.  In-place left shifts:
                per-partition masks mean untouched rows keep their values;
                shifted reads are ahead of writes (stream order) so in-place
                is safe.  src=None means wb was pre-loaded by the caller.
                """
                nbits = (maxoff).bit_length()
                cw = out_w + maxoff
                if src is not None:
                    nc.vector.tensor_copy(wb[:, 0:cw], src[:, 0:cw])
                for b in range(nbits - 1, -1, -1):
                    sh = 1 << b
                    w = out_w + min(maxoff, 2 * sh - 1) - sh
                    nc.vector.copy_predicated(
                        wb[:, 0:w], bits[:, b:b + 1].to_broadcast((P, w)),
                        wb[:, sh:sh + w])

            def mkbits(src_i32):
                nc.vector.tensor_tensor(
                    bits[:], src_i32[:, 0:1].to_broadcast((P, 8)), i8_t[:],
                    Op.logical_shift_right)
                nc.vector.tensor_tensor(bits[:], bits[:], onesI[:],
                                        Op.bitwise_and)

            for it in range(THINK_ITER):
                # --- dot[p,s] = sum_c xpad[p,s+c]*yres[p,c]: one MAC-scan,
                # windowed sums recovered by differencing page ends ---
                in0 = bass.AP(xpad.tensor, xpad.offset,
                              [list(xpad.ap[0]), [1, S1], [1, 80]])
                in1 = bass.AP(yres.tensor, yres.offset,
                              [list(yres.ap[0]), [0, S1], [1, 80]])
                dview = dscan[:].rearrange("p (s c) -> p s c", c=80)
                nc.vector._custom_dve(macscan, out=dview, in0=in0, in1=in1)
                pe0 = bass.AP(dscan.tensor, dscan.offset + 79, [list(dscan.ap[0]), [80, 158]])
                pe1 = bass.AP(dscan.tensor, dscan.offset + 159, [list(dscan.ap[0]), [80, 158]])
                nc.vector.tensor_tensor(dot[:, 1:159], pe1, pe0, Op.subtract)
                nc.vector.tensor_copy(dot[:, 0:1], dscan[:, 79:80])
                # --- theta = argmax dot*|dot|/nsq (fused custom op) ---
                nc.vector._custom_dve(signsq, out=gsel[:], in0=dot[:],
                                      in1=rnsq[:])
                nc.vector.max(mx8[:], gsel[:])
                nc.vector.max_index(mi8[:], mx8[:], gsel[:])
                nc.vector.tensor_copy(thf[:], mi8[:, 0:1])
                # --- y_align barrel gather: yal = wb1[:, 0:80] ---
                nc.vector.tensor_copy(offI[:], mi8[:, 0:1])
                mkbits(offI)
                barrel(None, wb1, 80, 158)
                yal = wb1
                # --- softmax attention -> y_att in yap[:, 80:160] ---
                # (no max-subtraction: |zt|/T is far inside fp32 exp range)
                nc.vector.tensor_tensor(zt[:], yal[:, 0:80], yres[:], Op.mult)
                nc.scalar.activation(et[:], zt[:], AF.Exp,
                                     scale=1.0 / TEMPER,
                                     accum_out=ssum[:, 0:1])
                nc.vector.reciprocal(rsum[:], ssum[:])
                nc.vector.scalar_tensor_tensor(yap[:, 80:160], et[:],
                                               rsum[:, 0:1], yal[:, 0:80],
                                               Op.mult, Op.mult)
                nc.scalar.copy(yah[:], yap[:, 80:240])
                # --- tail features: ya^T via DMA xbar transpose ---
                nc.scalar.dma_start_transpose(etl[:], yah[:, 0:128])
                # --- z features z[p, 80d+i] = ye[i]*ye[i+d], built in halves.
                # Transposes split three ways so they run in parallel:
                # chunks 0-20 xbar via sync queue, 21-41 xbar via scalar
                # queue, 42-49 via a PE transpose->copy pipeline. ---
                Eps = ppe.tile([P, S2], mybir.dt.float32, tag="Eps")
                half = NCHUNK // 2

                def zbuild(h):
                    zin0 = bass.AP(yah.tensor, yah.offset,
                                   [list(yah.ap[0]), [0, 40], [1, 80]])
                    zin1 = bass.AP(yah.tensor, yah.offset + 40 * h,
                                   [list(yah.ap[0]), [1, 40], [1, 80]])
                    zv = zfeat[:, h * half * 128:(h + 1) * half * 128
                               ].rearrange("p (d i) -> p d i", i=80)
                    nc.vector.tensor_tensor(zv, zin0, zin1, Op.mult)

                def xbarT(k0, k1, eng):
                    zo = zsb[:, k0 * 128:k1 * 128].rearrange(
                        "p (k t) -> p k t", t=128)
                    eng.dma_start_transpose(zo, zfeat[:, k0 * 128:k1 * 128])

                def emms(k0, k1):
                    for k in range(k0, k1):
                        nc.tensor.matmul(
                            Eps[:], zsb[:, k * 128:(k + 1) * 128],
                            A_t[:, k * 81:(k + 1) * 81],
                            start=(k == 0), stop=False)

                zbuild(0)
                xbarT(0, 21, nc.sync)
                zbuild(1)
                xbarT(21, 42, nc.scalar)
                for b0 in (42, 46):
                    zps = pp.tile([128, 512], BF16, tag="zps")
                    for q in range(4):
                        k = b0 + q
                        nc.tensor.transpose(zps[:, q * 128:(q + 1) * 128],
                                            zfeat[:, k * 128:(k + 1) * 128],
                                            id_t[:])
                    nc.scalar.copy(zsb[:, b0 * 128:(b0 + 4) * 128],
                                   zps[:])
                emms(0, 21)
                emms(21, 42)
                emms(42, NCHUNK)
                nc.tensor.matmul(Eps[:], etl[0:80, :], At_t[:], start=False,
                                 stop=True)
                # --- x_ele gather (overlaps E pipe): off = 159 - theta ---
                nc.vector.tensor_scalar(thxf[:], thf[:], -1.0, 159.0,
                                        Op.mult, Op.add)
                nc.vector.tensor_copy(offI[:], thxf[:])
                mkbits(offI)
                barrel(yap, wb2, 80, 159)
                nc.vector.tensor_tensor(xpad[:, 79:159], xpad[:, 79:159],
                                        wb2[:, 0:80], Op.subtract)
                # next-iter sliding norms (overlaps E pipe)
                sliding_norms()
                # --- s* = argmax E (straight from PSUM), d* = 80-s* ---
                nc.vector.max(mx8[:], Eps[:])
                nc.vector.max_index(mi8[:], mx8[:], Eps[:])
                nc.vector.tensor_copy(sff[:], mi8[:, 0:1])
                nc.vector.tensor_scalar(dff[:], sff[:], -1.0, 80.0,
                                        Op.mult, Op.add)
                # --- yhat barrel gather: off = s*, width 160 ---
                nc.vector.tensor_copy(offI[:], mi8[:, 0:1])
                mkbits(offI)
                barrel(yap, wb3, 160, 80)
                nc.scalar.copy(yhath[:], wb3[:, 0:160])
                # --- H = W_enc @ yhat^T + b_enc (h-major, bias folded) ---
                yh0p = pp.tile([128, 128], BF16, tag="tp")
                nc.tensor.transpose(yh0p[:], yhath[:, 0:128], id_t[:])
                nc.scalar.copy(yh0[:], yh0p[:])
                yh1p = pp.tile([128, 128], BF16, tag="tp")
                nc.tensor.transpose(yh1p[0:32, :], yhath[:, 128:160], id_t[:])
                nc.scalar.copy(yh1[0:32, :], yh1p[0:32, :])
                Hps = pph.tile([128, 512], mybir.dt.float32, tag="Hps")
                for hc in range(4):
                    r = slice(hc * 128, (hc + 1) * 128)
                    nc.tensor.matmul(Hps[:, r], We1_t[:, r], yh0[:],
                                     start=True, stop=False)
                    nc.tensor.matmul(Hps[:, r], We2_t[:, r], yh1[:],
                                     start=False, stop=True)
                nc.scalar.copy(hsb[:], Hps[:])
                # --- X = W_src @ h + b_src (token-major via flip) ---
                Xps = ppx.tile([128, 160], mybir.dt.float32, tag="Xps")
                for hc in range(4):
                    nc.tensor.matmul(Xps[:], hsb[:, hc * 128:(hc + 1) * 128],
                                     Ws_t[:, hc * 160:(hc + 1) * 160],
                                     start=(hc == 0), stop=False)
                nc.tensor.matmul(Xps[:], on_t[:], bs_t[:], start=False,
                                 stop=True)
                # --- y_ele barrel gather straight from PSUM: off = d* ---
                nc.vector.tensor_copy(offI[:], dff[:])
                mkbits(offI)
                barrel(Xps, wb2, 80, 80)
                yele = wb2
                # --- state update first (unblocks next iter's dot), then
                # loss from the identity  y_ele - y_res_old = -y_res_new ---
                nc.vector.tensor_tensor(yres[:], yres[:], yele[:, 0:80],
                                        Op.subtract)
                nc.vector._custom_dve(msqacc, out=dsq[:], in0=yres[:],
                                      in1=keep[:],
                                      accum_out=lossp[:, it:it + 1])

            nc.sync.dma_start(d_out[:], lossp[:])
    return nc


def kernel(x, y, W_enc, b_enc, W_src, b_src):
    import sys
    if '/opt/trn_rl_repo' not in sys.path:
        sys.path.insert(0, '/opt/trn_rl_repo')
    x = np.asarray(x, np.float32)
    y = np.asarray(y, np.float32)
    consts = _build_consts(W_enc, b_enc, W_src, b_src)

    if "nc" not in _cache:
        _cache["macscan"], _cache["signsq"], _cache["msqacc"] = _register_ops()
        _cache["nc"] = _build_nc()
        _cache["nc"].finalize()
    nc = _cache["nc"]

    xt = x.reshape(NTOK, IDIM)
    yt = y.reshape(NTOK, ODIM)
    in_maps = []
    for c in range(NCORES):
        m = dict(consts)
        m["x"] = np.ascontiguousarray(xt[c * P:(c + 1) * P])
        m["y"] = np.ascontiguousarray(yt[c * P:(c + 1) * P])
        in_maps.append(m)

    from concourse.bass_utils import run_bass_kernel_spmd
    res = run_bass_kernel_spmd(nc, in_maps, list(range(NCORES)))
    parts = np.stack([r["losspart"] for r in res.results])
    keep_cnt = max(int((y != 0.0).sum()), 1)
    nums = parts[:, :, :THINK_ITER].sum(axis=(0, 1), dtype=np.float64)
    losses = (nums / keep_cnt).astype(np.float32)
    return np.float32(np.mean(losses))


# revision 80
# speedup vs baseline: 1.2676x; 1.0976x over previous
"""Trainium2 Bass kernel for nn_Net_17532056502451.

5 "think" iterations: shift-window cosine selector (159 shifts) + softmax
attention + scatter-back + conv-style encoder/decoder with energy argmax
(81 shifts), masked-MSE losses averaged.  Data-parallel: 1024 tokens over
8 cores, 128 tokens/core (one per SBUF partition), token-major.

v2 mappings per core:
- dot correlation: ONE bf16 tensor_tensor (2 elem/cyc) building all 12720
  products + bf16 tree adds (80->40->20->10->5) + fp32 tensor_reduce.
- sliding norms: Square + prefix-scan + strided diff (fp32).
- per-token dynamic windows: GPSIMD ap_gather (idx[p,j] = off_p + j, 16
  candidate lanes) + mask-mult + tensor_reduce diagonal extract (2 DVE ops
  instead of 16 predicated copies).
- energy: quadratic Gram form, all-bf16 PE pipeline: z built in one DVE op,
  transposed on PE (bf16, 1 cyc/row), PSUM->SBUF copies batched 4 chunks at
  a time, E matmuls flipped (z chunk stationary, Az moving) so E lands
  token-major in PSUM - no transpose-back.
- encoder: h-major H matmuls with b_enc folded in via a ones row;
  decoder: X matmuls flipped token-major with b_src via a k=1 matmul -
  no back-transposes.
- softmax exp + denominator in one scalar-engine op (accum_out); loss
  sum-of-squares via scalar Square + accum_out.
"""
import numpy as np

IDIM = 80
ODIM = 80
HDIM = 512
THINK_ITER = 5
TEMPER = 0.7
B, T = 4, 256
NTOK = B * T
P = 128
NCORES = 8
S1 = 159
S2 = 81
NFEAT = 80 * 80
NCHUNK = NFEAT // 128   # 50

_cache = {}


def _bf16(a):
    import ml_dtypes
    return np.asarray(a, dtype=ml_dtypes.bfloat16)


def _register_ops():
    """Custom DVE ops, registered at runtime (documented extension point:
    the per-NEFF DVE table).

    MACSCAN_ANT: out[p,k] = running sum of in0[p,:k+1]*in1[p,:k+1] — one
    full-rate pass computes all windowed dot products; window sums are
    recovered by differencing page-end prefix values.
    SIGNSQ_ANT: out = in0*|in0|*in1 — the selector score in one op.
    """
    import concourse.dve_ops as dve_ops
    from concourse.dve_ops import DveOp, OPS, DveOpSpec
    from concourse.dve_spec import Spec, Src0, Src1, Zero, scan, lower, relu
    from concourse.dve_spec import AluOp as SAlu
    from concourse.dve_spec import _has_src1 as has_src1

    have = {o.name: o for o in OPS}
    if "MACSCAN_ANT" in have:
        return have["MACSCAN_ANT"], have["SIGNSQ_ANT"], have["MSQACC_ANT"]

    def ref_scan(in0, in1):
        p = in0.shape[0]
        prod = (np.asarray(in0, np.float64) * np.asarray(in1, np.float64))
        return np.cumsum(prod.reshape(p, -1), axis=1).reshape(
            in0.shape).astype(np.float32)

    def ref_signsq(in0, in1):
        return in0 * np.abs(in0) * in1

    def ref_msq(in0, in1):
        return (in0 * in1) ** 2

    from concourse.dve_spec import sq
    a = relu(Src0) + relu(Zero - Src0)
    specs = [("MACSCAN_ANT", Spec(body=scan(SAlu.ADD, Src0 * Src1),
                                  reference=ref_scan)),
             ("SIGNSQ_ANT", Spec(body=Src0 * a * Src1, reference=ref_signsq)),
             ("MSQACC_ANT", Spec(body=sq(Src0 * Src1), accum=SAlu.ADD,
                                 reference=ref_msq))]
    out = []
    for name, sp in specs:
        row = max(dve_ops._SUB_OPCODE_FOR_NAME.values()) + 1
        dve_ops._SUB_OPCODE_FOR_NAME[name] = row
        shas = {}
        for ver in ("v3", "v4"):
            tmp = DveOpSpec(name=name, opcode=row, uops=lower(sp, ver=ver),
                            rd1_en=has_src1(sp))
            shas[ver] = tmp.sha(ver)
        op = DveOp(name, sp, subdim=False, uops_sha=shas)
        OPS.append(op)
        dve_ops.CUSTOM_DVE_SPECS[op.name] = op.spec
        out.append(op)
    return tuple(out)


def _build_consts(W_enc, b_enc, W_src, b_src):
    W_enc = np.asarray(W_enc, np.float32)
    b_enc = np.asarray(b_enc, np.float32)
    W_src = np.asarray(W_src, np.float32)
    b_src = np.asarray(b_src, np.float32)
    C = (W_enc.T @ W_enc).astype(np.float32)
    q = (W_enc.T @ b_enc).astype(np.float32)
    bb = np.float32(b_enc @ b_enc)
    # E[t,s] = sum_{d,i} Az[s, 80d+i] * ye_i ye_{i+d} + sum_i 2 q[dd+i] ye_i
    #          + bb,  dd = 80 - s
    Az = np.zeros((S2, NFEAT), np.float32)
    Al = np.zeros((S2, 80), np.float32)
    for s in range(S2):
        dd = 80 - s
        blk = C[dd:dd + 80, dd:dd + 80]
        for d in range(80):
            diag = np.diagonal(blk, offset=d).copy()
            Az[s, d * 80: d * 80 + (80 - d)] = (2.0 if d > 0 else 1.0) * diag
        Al[s, :80] = 2.0 * q[dd:dd + 80]
        # bb (constant across s and t) dropped: argmax-invariant shift
    Az_cat = np.ascontiguousarray(Az.T)               # (6400, 81): pi-major
    Atail = np.ascontiguousarray(Al.T)                # (80, 81)
    # decoder is linear end-to-end: fold W_src @ W_enc and the biases
    Wf = (W_src @ W_enc).astype(np.float32)           # (160, 160)
    bf = (W_src @ b_enc + b_src).astype(np.float32)   # (160,)
    WfT = np.ascontiguousarray(Wf.T)                  # (160 c, 160 o)
    Wf1 = WfT[0:128]
    Wf2 = np.zeros((33, 160), np.float32)             # 32 c's + bias row
    Wf2[0:32] = WfT[128:160]
    Wf2[32] = bf
    iota8 = np.broadcast_to(np.arange(8, dtype=np.int32), (P, 8)).copy()
    ident = np.eye(128, dtype=np.float32)
    return dict(Az=_bf16(Az_cat), Atail=_bf16(Atail),
                Wf1=_bf16(Wf1), Wf2=_bf16(Wf2),
                ones1=_bf16(np.ones((1, 128), np.float32)),
                ident=_bf16(ident), iota8=iota8)


def _build_nc():
    import concourse.bass as bass
    import concourse.bacc as bacc
    import concourse.mybir as mybir
    from concourse.tile import TileContext

    F32 = mybir.dt.float32
    BF16 = mybir.dt.bfloat16
    I16 = mybir.dt.int16
    U32 = mybir.dt.uint32
    Op = mybir.AluOpType
    AF = mybir.ActivationFunctionType
    AX = mybir.AxisListType

    nc = bacc.Bacc()
    d_x = nc.declare_dram_parameter("x", [P, 80], F32, isOutput=False)
    d_y = nc.declare_dram_parameter("y", [P, 80], F32, isOutput=False)
    d_A = nc.declare_dram_parameter("Az", [NFEAT, 81], BF16, isOutput=False)
    d_At = nc.declare_dram_parameter("Atail", [80, 81], BF16, isOutput=False)
    d_Wf1 = nc.declare_dram_parameter("Wf1", [128, 160], BF16, isOutput=False)
    d_Wf2 = nc.declare_dram_parameter("Wf2", [33, 160], BF16, isOutput=False)
    d_on = nc.declare_dram_parameter("ones1", [1, 128], BF16, isOutput=False)
    d_id = nc.declare_dram_parameter("ident", [128, 128], BF16, isOutput=False)
    d_i8 = nc.declare_dram_parameter("iota8", [P, 8], mybir.dt.int32,
                                     isOutput=False)
    d_out = nc.declare_dram_parameter("losspart", [P, 8], F32, isOutput=True)

    with TileContext(nc) as tc:
        with (
            tc.tile_pool(name="const", bufs=1) as cpool,
            tc.tile_pool(name="work", bufs=1) as pool,
            tc.tile_pool(name="ps_rot", bufs=2, space="PSUM") as pp,
            tc.tile_pool(name="ps_e", bufs=1, space="PSUM") as ppe,
            tc.tile_pool(name="ps_h", bufs=1, space="PSUM") as pph,
            tc.tile_pool(name="ps_x", bufs=1, space="PSUM") as ppx,
        ):
            # ---- inputs first: the first iteration only needs x/y ----
            xpad = pool.tile([P, 238], F32, tag="xpad")
            yres = pool.tile([P, 80], F32, tag="yres")
            nc.vector.memset(xpad[:], 0.0)
            nc.sync.dma_start(xpad[:, 79:159], d_x[:])
            nc.sync.dma_start(yres[:], d_y[:])
            # ---- constants ----
            A_t = cpool.tile([P, NCHUNK * 81], BF16, tag="A")
            for k in range(NCHUNK):
                nc.sync.dma_start(A_t[:, k * 81:(k + 1) * 81],
                                  d_A[k * 128:(k + 1) * 128, :])
            At_t = cpool.tile([80, 81], BF16, tag="At")
            nc.sync.dma_start(At_t[:], d_At[:])
            Wf1_t = cpool.tile([128, 160], BF16, tag="Wf1")
            nc.sync.dma_start(Wf1_t[:], d_Wf1[:])
            Wf2_t = cpool.tile([33, 160], BF16, tag="Wf2")
            nc.sync.dma_start(Wf2_t[:], d_Wf2[:])
            id_t = cpool.tile([128, 128], BF16, tag="id")
            nc.sync.dma_start(id_t[:], d_id[:])
            i8_t = cpool.tile([P, 8], mybir.dt.int32, tag="i8")
            nc.sync.dma_start(i8_t[:], d_i8[:])

            # ---- state ----
            keep = pool.tile([P, 80], F32, tag="keep")
            yap = pool.tile([P, 240], F32, tag="yap")
            lossp = pool.tile([P, 8], F32, tag="lossp")
            yah = pool.tile([P, 160], BF16, tag="yah")
            nc.vector.memset(yap[:], 0.0)
            nc.vector.memset(lossp[:], 0.0)
            nc.vector.tensor_scalar(keep[:], yres[:], 0.0, None, Op.not_equal)

            # dot workspace
            dscan = pool.tile([P, S1 * 80], F32, tag="dscan")
            dot = pool.tile([P, S1], F32, tag="dot")
            gsel = pool.tile([P, S1], F32, tag="gsel")
            cs2 = pool.tile([P, 238], F32, tag="cs2")
            nsq = pool.tile([P, S1], F32, tag="nsq")
            rnsq = pool.tile([P, S1], F32, tag="rnsq")

            I32 = mybir.dt.int32
            mx8 = pool.tile([P, 8], F32, tag="mx8")
            mi8 = pool.tile([P, 8], U32, tag="mi8")
            thf = pool.tile([P, 1], F32, tag="thf")
            thxf = pool.tile([P, 1], F32, tag="thxf")
            sff = pool.tile([P, 1], F32, tag="sff")
            dff = pool.tile([P, 1], F32, tag="dff")
            offI = pool.tile([P, 1], I32, tag="offI")
            bits = pool.tile([P, 8], I32, tag="bits")
            onesI = pool.tile([P, 8], I32, tag="onesI")
            nc.vector.memset(onesI[:], 1)
            wb1 = pool.tile([P, 240], F32, tag="wb1")
            wb2 = pool.tile([P, 240], F32, tag="wb2")
            wb3 = pool.tile([P, 240], F32, tag="wb3")
            yhath = pool.tile([P, 160], BF16, tag="yhath")
            zt = pool.tile([P, 80], F32, tag="zt")
            et = pool.tile([P, 80], F32, tag="et")
            ssum = pool.tile([P, 1], F32, tag="ssum")
            rsum = pool.tile([P, 1], F32, tag="rsum")
            nzm = pool.tile([P, 1], F32, tag="nzm")

            zfeat = pool.tile([P, NFEAT], BF16, tag="zfeat")
            zsb = pool.tile([P, NFEAT], BF16, tag="zsb")
            etl = pool.tile([128, 128], BF16, tag="etl")
            yh0 = pool.tile([128, 128], BF16, tag="yh0")
            yh1 = pool.tile([33, 128], BF16, tag="yh1")
            nc.sync.dma_start(yh1[32:33, :], d_on[:])

            dtmp = pool.tile([P, 80], F32, tag="dtmp")
            dsq = pool.tile([P, 80], F32, tag="dsq")

            macscan = _cache["macscan"]
            signsq = _cache["signsq"]
            msqacc = _cache["msqacc"]

            def sliding_norms():
                # cs2[j] = sum_{m<=j} xpad[m]^2; nsq[s] = cs2[s+79]-cs2[s-1];
                # rnsq = 1/max(nsq, eps) precomputed off the critical path
                nc.vector._custom_dve(macscan, out=cs2[:], in0=xpad[:],
                                      in1=xpad[:])
                nc.vector.tensor_tensor(nsq[:, 1:159], cs2[:, 80:238],
                                        cs2[:, 0:158], Op.subtract)
                nc.vector.tensor_copy(nsq[:, 0:1], cs2[:, 79:80])
                nc.vector.tensor_scalar_max(rnsq[:], nsq[:], 1e-30)
                nc.vector.reciprocal(rnsq[:], rnsq[:])
                # prefetch the y_align barrel source
                nc.vector.tensor_copy(wb1[:, 0:238], xpad[:])

            sliding_norms()

            def barrel(src, wb, out_w, maxoff):
                """wb[p, j] <- src[p, off_p + j] for j in [0, out_w).

                bits must already hold (off >> b) & 1.  In-place left shifts:
                per-partition masks mean untouched rows keep their values;
                shifted reads are ahead of writes (stream order) so in-place
                is safe.  src=None means wb was pre-loaded by the caller.
                """
                nbits = (maxoff).bit_length()
                cw = out_w + maxoff
                if src is not None:
                    nc.vector.tensor_copy(wb[:, 0:cw], src[:, 0:cw])
                for b in range(nbits - 1, -1, -1):
                    sh = 1 << b
                    w = out_w + min(maxoff, 2 * sh - 1) - sh
                    nc.vector.copy_predicated(
                        wb[:, 0:w], bits[:, b:b + 1].to_broadcast((P, w)),
                        wb[:, sh:sh + w])

            def mkbits(src_i32):
                nc.vector.tensor_tensor(
                    bits[:], src_i32[:, 0:1].to_broadcast((P, 8)), i8_t[:],
                    Op.logical_shift_right)
                nc.vector.tensor_tensor(bits[:], bits[:], onesI[:],
                                        Op.bitwise_and)

            for it in range(THINK_ITER):
                # --- dot[p,s] = sum_c xpad[p,s+c]*yres[p,c]: one MAC-scan,
                # windowed sums recovered by differencing page ends ---
                in0 = bass.AP(xpad.tensor, xpad.offset,
                              [list(xpad.ap[0]), [1, S1], [1, 80]])
                in1 = bass.AP(yres.tensor, yres.offset,
                              [list(yres.ap[0]), [0, S1], [1, 80]])
                dview = dscan[:].rearrange("p (s c) -> p s c", c=80)
                nc.vector._custom_dve(macscan, out=dview, in0=in0, in1=in1)
                pe0 = bass.AP(dscan.tensor, dscan.offset + 79, [list(dscan.ap[0]), [80, 158]])
                pe1 = bass.AP(dscan.tensor, dscan.offset + 159, [list(dscan.ap[0]), [80, 158]])
                nc.vector.tensor_tensor(dot[:, 1:159], pe1, pe0, Op.subtract)
                nc.vector.tensor_copy(dot[:, 0:1], dscan[:, 79:80])
                # --- theta = argmax dot*|dot|/nsq (fused custom op) ---
                nc.vector._custom_dve(signsq, out=gsel[:], in0=dot[:],
                                      in1=rnsq[:])
                nc.vector.max(mx8[:], gsel[:])
                nc.vector.max_index(mi8[:], mx8[:], gsel[:])
                nc.vector.tensor_copy(thf[:], mi8[:, 0:1])
                # --- y_align barrel gather: yal = wb1[:, 0:80] ---
                nc.vector.tensor_copy(offI[:], mi8[:, 0:1])
                mkbits(offI)
                barrel(None, wb1, 80, 158)
                yal = wb1
                # --- softmax attention -> y_att in yap[:, 80:160] ---
                # (no max-subtraction: |zt|/T is far inside fp32 exp range)
                nc.vector.tensor_tensor(zt[:], yal[:, 0:80], yres[:], Op.mult)
                nc.scalar.activation(et[:], zt[:], AF.Exp,
                                     scale=1.0 / TEMPER,
                                     accum_out=ssum[:, 0:1])
                nc.vector.reciprocal(rsum[:], ssum[:])
                nc.vector.scalar_tensor_tensor(yap[:, 80:160], et[:],
                                               rsum[:, 0:1], yal[:, 0:80],
                                               Op.mult, Op.mult)
                nc.scalar.copy(yah[:], yap[:, 80:240])
                # --- tail features: ya^T via DMA xbar transpose ---
                nc.scalar.dma_start_transpose(etl[:], yah[:, 0:128])
                # --- z features z[p, 80d+i] = ye[i]*ye[i+d], built in halves.
                # Transposes split three ways so they run in parallel:
                # chunks 0-20 xbar via sync queue, 21-41 xbar via scalar
                # queue, 42-49 via a PE transpose->copy pipeline. ---
                Eps = ppe.tile([P, S2], mybir.dt.float32, tag="Eps")
                half = NCHUNK // 2

                def zbuild(h):
                    zin0 = bass.AP(yah.tensor, yah.offset,
                                   [list(yah.ap[0]), [0, 40], [1, 80]])
                    zin1 = bass.AP(yah.tensor, yah.offset + 40 * h,
                                   [list(yah.ap[0]), [1, 40], [1, 80]])
                    zv = zfeat[:, h * half * 128:(h + 1) * half * 128
                               ].rearrange("p (d i) -> p d i", i=80)
                    nc.vector.tensor_tensor(zv, zin0, zin1, Op.mult)

                def xbarT(k0, k1, eng):
                    zo = zsb[:, k0 * 128:k1 * 128].rearrange(
                        "p (k t) -> p k t", t=128)
                    eng.dma_start_transpose(zo, zfeat[:, k0 * 128:k1 * 128])

                def emms(k0, k1):
                    for k in range(k0, k1):
                        nc.tensor.matmul(
                            Eps[:], zsb[:, k * 128:(k + 1) * 128],
                            A_t[:, k * 81:(k + 1) * 81],
                            start=(k == 0), stop=False)

                zbuild(0)
                xbarT(0, 25, nc.sync)
                zbuild(1)
                xbarT(25, 42, nc.scalar)
                zpss = {}
                for b0 in (42, 46):
                    zps = pp.tile([128, 512], BF16, tag="zps")
                    zpss[b0] = zps
                    for q in range(4):
                        k = b0 + q
                        nc.tensor.transpose(zps[:, q * 128:(q + 1) * 128],
                                            zfeat[:, k * 128:(k + 1) * 128],
                                            id_t[:])
                # --- x_ele gather (overlaps E pipe): off = 159 - theta ---
                nc.vector.tensor_scalar(thxf[:], thf[:], -1.0, 159.0,
                                        Op.mult, Op.add)
                nc.vector.tensor_copy(offI[:], thxf[:])
                mkbits(offI)
                barrel(yap, wb2, 80, 159)
                nc.vector.tensor_tensor(xpad[:, 79:159], xpad[:, 79:159],
                                        wb2[:, 0:80], Op.subtract)
                for b0 in (42, 46):
                    nc.vector.tensor_copy(zsb[:, b0 * 128:(b0 + 4) * 128],
                                          zpss[b0])
                emms(0, 25)
                emms(42, NCHUNK)
                emms(25, 42)
                nc.tensor.matmul(Eps[:], etl[0:80, :], At_t[:], start=False,
                                 stop=True)
                # next-iter sliding norms (overlaps E pipe)
                sliding_norms()
                # --- s* = argmax E (straight from PSUM), d* = 80-s* ---
                nc.vector.max(mx8[:], Eps[:])
                nc.vector.max_index(mi8[:], mx8[:], Eps[:])
                nc.vector.tensor_copy(sff[:], mi8[:, 0:1])
                nc.vector.tensor_scalar(dff[:], sff[:], -1.0, 80.0,
                                        Op.mult, Op.add)
                # --- yhat barrel gather: off = s*, width 160 ---
                nc.vector.tensor_copy(offI[:], mi8[:, 0:1])
                mkbits(offI)
                barrel(yap, wb3, 160, 80)
                nc.scalar.copy(yhath[:], wb3[:, 0:160])
                # --- fused decoder: x_ext = (W_src W_enc) yhat + bfused,
                # token-major; bias via the ones row of yh1 ---
                yh0p = pp.tile([128, 128], BF16, tag="tp")
                nc.tensor.transpose(yh0p[:], yhath[:, 0:128], id_t[:])
                nc.scalar.copy(yh0[:], yh0p[:])
                yh1p = pp.tile([128, 128], BF16, tag="tp")
                nc.tensor.transpose(yh1p[0:32, :], yhath[:, 128:160], id_t[:])
                nc.scalar.copy(yh1[0:32, :], yh1p[0:32, :])
                Xps = ppx.tile([128, 160], mybir.dt.float32, tag="Xps")
                nc.tensor.matmul(Xps[:], yh0[:], Wf1_t[:],
                                 start=True, stop=False)
                nc.tensor.matmul(Xps[:], yh1[:], Wf2_t[:],
                                 start=False, stop=True)
                # --- y_ele barrel gather straight from PSUM: off = d* ---
                nc.vector.tensor_copy(offI[:], dff[:])
                mkbits(offI)
                barrel(Xps, wb2, 80, 80)
                yele = wb2
                # --- state update first (unblocks next iter's dot), then
                # loss from the identity  y_ele - y_res_old = -y_res_new ---
                nc.vector.tensor_tensor(yres[:], yres[:], yele[:, 0:80],
                                        Op.subtract)
                nc.vector._custom_dve(msqacc, out=dsq[:], in0=yres[:],
                                      in1=keep[:],
                                      accum_out=lossp[:, it:it + 1])

            nc.sync.dma_start(d_out[:], lossp[:])
    return nc


def kernel(x, y, W_enc, b_enc, W_src, b_src):
    import sys
    if '/opt/trn_rl_repo' not in sys.path:
        sys.path.insert(0, '/opt/trn_rl_repo')
    x = np.asarray(x, np.float32)
    y = np.asarray(y, np.float32)
    consts = _build_consts(W_enc, b_enc, W_src, b_src)

    if "nc" not in _cache:
        _cache["macscan"], _cache["signsq"], _cache["msqacc"] = _register_ops()
        _cache["nc"] = _build_nc()
        _cache["nc"].finalize()
    nc = _cache["nc"]

    xt = x.reshape(NTOK, IDIM)
    yt = y.reshape(NTOK, ODIM)
    in_maps = []
    for c in range(NCORES):
        m = dict(consts)
        m["x"] = np.ascontiguousarray(xt[c * P:(c + 1) * P])
        m["y"] = np.ascontiguousarray(yt[c * P:(c + 1) * P])
        in_maps.append(m)

    from concourse.bass_utils import run_bass_kernel_spmd
    res = run_bass_kernel_spmd(nc, in_maps, list(range(NCORES)))
    parts = np.stack([r["losspart"] for r in res.results])
    keep_cnt = max(int((y != 0.0).sum()), 1)
    nums = parts[:, :, :THINK_ITER].sum(axis=(0, 1), dtype=np.float64)
    losses = (nums / keep_cnt).astype(np.float32)
    return np.float32(np.mean(losses))
